# revision 33
# baseline (speedup 1.0000x reference)
"""Trainium2 Bass kernel for gated GQA attention (nn_Attention_6476810683032).

Sharding: 8 cores = 2 (batch DP) x 4 (head-group TP).
Core c handles batch b=c//4, head group g=c%4 (q-heads 4g..4g+3, kv-head g).
Each core computes a partial o_proj output [D, T] (its 4 heads' contribution,
transposed layout); the host sums the 4 partials per batch and transposes.

On-device per core (all matmuls bf16 with fp32 PSUM accumulation):
  - projections from host-pre-transposed hidden_t [D, T] (channel-major
    outputs for q/gate/k, token-major for v) -- no on-device transposes
  - RMS norm via ones-matmul partition reduction + K=1 broadcast matmul
  - RoPE via partition-offset elementwise ops with a pre-signed sin table
  - causal attention in transposed-score form: S_T[tk,tq] = k_rot.T@q_rot,
    exp without max subtraction (logits bounded by the RMS norms)
  - softmax denominator: E tiles accumulate in SBUF on the DVE (bf16
    ping-pong adds stay in the 2x packed mode; partial-width diagonal
    steps add in place), then ONE ones-matmul per (chunk, head) -- saves
    ~60k PE columns (~30us) vs a per-m-step ones-matmul
  - sigmoid folded into the normalization: attn*sig(g)/den ==
    attn/((1+e^-g)*den); e^-g shares the exp ACT table, so the whole
    kernel uses Exp + Abs_reciprocal_sqrt only (9 table loads; eg exps
    are dep-ordered after the chunk's absr chain to prevent interleave)
  - partial o_proj: out_T[dout,t] = wo_slice.T @ gated (bf16 partials,
    summed in f32 on the host); the last chunk's heads-0/1 half goes to
    out2 as PE filler during the pair-(2,3) m-loop

Scheduling notes (PE p-state drops to 1.2GHz after any bubble and takes
~3us of continuous execution to return to 2.4GHz, so PE density is worth
more than engine-local optima; the chip also thermally throttles ~18%
run-to-run -- compare cold first-exec numbers or LDWEIGHTS-normalized):
  - hid loads in half-T tiles; pass A (k/v chunks 0-1 + head-0 q/gate)
    paces with the half-0 stripe; wqq/wqg are head-major 32KB tiles so
    each head's projection starts as its 0.5MB block lands
  - chunk 0 is fully reordered around the DMA stream: q-projections +
    all chains (wqq in-stripe), pair-(0,1) m-loop, pass B (k/v chunks
    2-3 from half 1), gate projections (wqg lands last), gating drains
  - input DMAs issue in first-use order, striped across the three rings
    (sync/scalar HWDGE + gpsimd SWDGE) balanced by bytes
  - o_proj of chunk c-1 + chunk-1 prefetch + last-chunk half o_proj are
    drip-fed between m-steps and into the drains as PE filler
  - final o_proj drains through 4 osb buffers, casts split DVE/ACT,
    writes striped across all three DMA rings
Measured: 419us baseline -> 376us (cold-clock best; ~81% PE occupancy).
"""

import os
import sys
from contextlib import ExitStack

import numpy as np

sys.path.insert(0, "/opt/trn_rl_repo")

import ml_dtypes  # noqa: E402

import concourse.bass as bass  # noqa: E402
import concourse.mybir as mybir  # noqa: E402
import concourse.tile as tile  # noqa: E402
from concourse import bacc  # noqa: E402
from concourse import masks as masks_mod  # noqa: E402

F32 = mybir.dt.float32
BF16 = mybir.dt.bfloat16
AF = mybir.ActivationFunctionType
ALU = mybir.AluOpType
BF = ml_dtypes.bfloat16

P = 128
B, T, D = 2, 2048, 2048
NH, NKV, HD = 16, 4, 128
NHL = NH // NKV          # local q heads per core (4)
CH = 4                   # tq chunks
CT = T // CH             # 512 tokens per chunk
TH = T // 2              # half-T (hid DMA tile width)
DT = D // P              # 16 contraction tiles
KT = T // P              # 16 tk tiles
EPS = 1e-6
SCALE = HD ** -0.5
N_CORES = 8


def _norm_rope(nc, pools, psr, pss, ones_col, ones_row, eps_t, x_bf, w_ap,
               cos_sl, sin_sl, out_ap, n):
    """RMS-norm (over partitions) + RoPE on a [128, n] channel-major tile.

    x_bf: [128, n] bf16 SBUF (pre-norm channels-on-partitions tile)
    w_ap: [128, 1] f32 norm weight
    cos_sl/sin_sl: [128, n] bf16 (sin pre-signed: rows 0-63 negated)
    out_ap: [128, n] bf16 destination
    """
    sbw, sbr = pools
    xsq = sbw.tile([P, n], BF16, tag="tmpa", name="xsq")
    nc.vector.tensor_tensor(xsq[:], x_bf, x_bf, op=ALU.mult)
    ssq = psr.tile([1, n], F32, tag="row", name="ssq")
    nc.tensor.matmul(ssq[:], ones_col, xsq[:], start=True, stop=True)
    rsq = sbr.tile([1, n], BF16, tag="rsq", name="rsq")
    absr = nc.scalar.activation(rsq[:], ssq[:], AF.Abs_reciprocal_sqrt,
                                scale=1.0 / HD, bias=eps_t)
    rb = pss.tile([P, n], F32, tag="ss", name="rb")
    nc.tensor.matmul(rb[:], ones_row, rsq[:], start=True, stop=True)
    wr = sbw.tile([P, n], BF16, tag="tmpb", name="wr")
    nc.vector.tensor_scalar(wr[:], rb[:], w_ap, None, ALU.mult)
    xn = sbw.tile([P, n], BF16, tag="xn", name="xn")
    nc.vector.tensor_tensor(xn[:], x_bf, wr[:], op=ALU.mult)
    t1 = sbw.tile([P, n], BF16, tag="tmpb", name="t1")
    nc.vector.tensor_tensor(t1[:], xn[:], cos_sl, op=ALU.mult)
    h = HD // 2
    xs = sbw.tile([P, n], BF16, tag="tmpc", name="xs", bufs=2)
    nc.vector.tensor_copy(xs[0:h, :], xn[h:P, :])
    nc.vector.tensor_copy(xs[h:P, :], xn[0:h, :])
    t2 = sbw.tile([P, n], BF16, tag="tmpa", name="t2")
    nc.vector.tensor_tensor(t2[:], xs[:], sin_sl, op=ALU.mult)
    nc.vector.tensor_tensor(out_ap, t1[:], t2[:], op=ALU.add)
    return absr


def build_nc():
    nc = bacc.Bacc("TRN2", target_bir_lowering=False, debug=False,
                   num_devices=N_CORES)
    hid_d = nc.dram_tensor("hid", [D, T], BF16, kind="ExternalInput")
    wqq_d = nc.dram_tensor("wqq", [NHL * D, HD], BF16, kind="ExternalInput")
    wqg_d = nc.dram_tensor("wqg", [NHL * D, HD], BF16, kind="ExternalInput")
    wk_d = nc.dram_tensor("wk", [D, HD], BF16, kind="ExternalInput")
    wv_d = nc.dram_tensor("wv", [D, HD], BF16, kind="ExternalInput")
    wo_d = nc.dram_tensor("wo", [NHL * HD, D], BF16, kind="ExternalInput")
    cos_d = nc.dram_tensor("cost", [P, T], BF16, kind="ExternalInput")
    sin_d = nc.dram_tensor("sinpm", [P, T], BF16, kind="ExternalInput")
    qw_d = nc.dram_tensor("qw", [P, 1], F32, kind="ExternalInput")
    kw_d = nc.dram_tensor("kw", [P, 1], F32, kind="ExternalInput")
    mask_d = nc.dram_tensor("masks", [P, P], BF16, kind="ExternalInput")
    out_d = nc.dram_tensor("out_t", [D, T], BF16, kind="ExternalOutput")
    # heads 0-1's half of the last chunk's o_proj (host adds the two halves)
    out2_d = nc.dram_tensor("out2", [D, CT], BF16, kind="ExternalOutput")

    with tile.TileContext(nc) as tc, ExitStack() as ctx, \
            nc.allow_low_precision(reason="bf16 softmax temps validated by rel_err"):
        sbp = ctx.enter_context(tc.tile_pool(name="sbp", bufs=1))
        sbw = ctx.enter_context(tc.tile_pool(name="sbw", bufs=3))
        sbr = ctx.enter_context(tc.tile_pool(name="sbr", bufs=2))
        sbq = ctx.enter_context(tc.tile_pool(name="sbq", bufs=6))
        psp = ctx.enter_context(tc.tile_pool(name="psp", bufs=2, space="PSUM"))
        pss = ctx.enter_context(tc.tile_pool(name="pss", bufs=2, space="PSUM"))
        psa = ctx.enter_context(tc.tile_pool(name="psa", bufs=2, space="PSUM"))
        psr = ctx.enter_context(tc.tile_pool(name="psr", bufs=2, space="PSUM"))

        # ---- persistent tiles ----
        wkT = [sbp.tile([P, HD], BF16, tag=f"wk{i}", name=f"wk{i}")
               for i in range(DT)]
        wvT = [sbp.tile([P, HD], BF16, tag=f"wv{i}", name=f"wv{i}")
               for i in range(DT)]
        hidh = [[sbp.tile([P, TH], BF16, tag=f"hid{d}_{j}", name=f"hid{d}_{j}")
                 for j in range(2)] for d in range(DT)]
        wqqt = [[sbp.tile([P, HD], BF16, tag=f"wqq{h}_{d}",
                         name=f"wqq{h}_{d}") for d in range(DT)]
                for h in range(NHL)]
        wqgt = [[sbp.tile([P, HD], BF16, tag=f"wqg{h}_{d}",
                         name=f"wqg{h}_{d}") for d in range(DT)]
                for h in range(NHL)]
        wo = [sbp.tile([P, D], BF16, tag=f"wo{i}", name=f"wo{i}")
              for i in range(NHL)]
        cost = sbp.tile([P, T], BF16, tag="cost")
        sinpm = sbp.tile([P, T], BF16, tag="sinpm")
        masks = sbp.tile([P, P], BF16, tag="masks")
        qw = sbp.tile([P, 1], F32, tag="qw")
        kw = sbp.tile([P, 1], F32, tag="kw")

        def hidsl(d, c):
            j, o = divmod(c, 2)
            return hidh[d][j][:, o * CT:(o + 1) * CT]

        # ---- DMA issue: first-use order, striped across the 3 rings ----
        rings = [nc.sync, nc.scalar, nc.gpsimd]
        ring_bytes = [0, 0, 0]

        def load(dst, src):
            i = ring_bytes.index(min(ring_bytes))
            rings[i].dma_start(dst, src)
            ring_bytes[i] += dst.size() * 2

        nc.gpsimd.dma_start(qw[:], qw_d[:, :])
        nc.gpsimd.dma_start(kw[:], kw_d[:, :])
        nc.gpsimd.dma_start(masks[:], mask_d[:, :])
        ring_bytes[2] += 64 * 1024
        for d in range(DT):
            ds_ = slice(d * P, (d + 1) * P)
            load(wkT[d][:], wk_d[ds_, :])
            load(wvT[d][:], wv_d[ds_, :])
            # split each hid tile across all 3 rings so per-ring FIFO
            # skew can't stall the d-ordered pass-A consumption
            for pi, (p0, p1) in enumerate(((0, 43), (43, 86), (86, 128))):
                rings[pi].dma_start(hidh[d][0][p0:p1, :],
                                    hid_d[d * P + p0:d * P + p1, 0:TH])
                ring_bytes[pi] += (p1 - p0) * TH * 2
            # head-0 q/gate weights ride the stripe (pass A needs them)
            load(wqqt[0][d][:], wqq_d[d * P:(d + 1) * P, :])
            load(wqgt[0][d][:], wqg_d[d * P:(d + 1) * P, :])
            # rope-table chunk slices just ahead of their chains' first use
            if d == 3:
                load(cost[:, 0:CT], cos_d[:, 0:CT])
                load(sinpm[:, 0:CT], sin_d[:, 0:CT])
            if d == 11:
                load(cost[:, CT:TH], cos_d[:, CT:TH])
                load(sinpm[:, CT:TH], sin_d[:, CT:TH])
        for h in range(1, NHL):
            for d in range(DT):
                ds_ = slice((h * D + d * P), (h * D + (d + 1) * P))
                load(wqqt[h][d][:], wqq_d[ds_, :])
        for d in range(DT):
            ds_ = slice(d * P, (d + 1) * P)
            load(hidh[d][1][:], hid_d[ds_, TH:T])
        for h in range(1, NHL):
            for d in range(DT):
                ds_ = slice((h * D + d * P), (h * D + (d + 1) * P))
                load(wqgt[h][d][:], wqg_d[ds_, :])
        load(cost[:, TH:T], cos_d[:, TH:T])
        load(sinpm[:, TH:T], sin_d[:, TH:T])
        for i in range(NHL):
            load(wo[i][:], wo_d[i * P:(i + 1) * P, :])

        ones_col = sbp.tile([P, 1], BF16, tag="ones_col")
        nc.vector.memset(ones_col[:], 1.0)
        ones_row = sbp.tile([1, P], BF16, tag="ones_row")
        nc.vector.memset(ones_row[:], 1.0)
        eps_t = sbp.tile([1, 1], F32, tag="eps_t")
        nc.vector.memset(eps_t[:], EPS)
        ident = sbp.tile([P, P], BF16, tag="ident")
        masks_mod.make_identity(nc, ident[:])
        krot = sbp.tile([P, T], BF16, tag="krot")
        vct = sbp.tile([P, T], BF16, tag="vct")
        vsb = []
        for i in range(KT):
            vsb.append(sbp.tile([P, HD], BF16, tag=f"v{i}", name=f"v{i}"))

        kbfs = []

        # ---- phase 0 pass A: k/v chunks 0-1 + chunk-0 q/gate, d-outer so
        # the PE paces with the hid half-0 DMA stream.
        def proj_pass(half, chunks, qg_psum):
            kps = [pss.tile([P, CT], F32, tag="ss", name=f"kps{c}")
                   for c in chunks]
            vps = [psr.tile([P, CT], F32, tag="row", name=f"vps{c}")
                   for c in chunks]
            for d in range(DT):
                st, sp = (d == 0), (d == DT - 1)
                for i, c in enumerate(chunks):
                    nc.tensor.matmul(kps[i][:], wkT[d][:, :], hidsl(d, c),
                                     start=st, stop=sp)
                    nc.tensor.matmul(vps[i][:], wvT[d][:, :], hidsl(d, c),
                                     start=st, stop=sp)
                if qg_psum is not None:
                    nc.tensor.matmul(qg_psum[0][:], wqqt[0][d][:],
                                     hidsl(d, 0), start=st, stop=sp)
                    nc.tensor.matmul(qg_psum[1][:], wqgt[0][d][:],
                                     hidsl(d, 0), start=st, stop=sp)
            for i, c in enumerate(chunks):
                kbf = sbw.tile([P, CT], BF16, tag="kbf", name="kbf", bufs=4)
                nc.vector.tensor_copy(kbf[:], kps[i][:])
                kbfs.append(kbf)
                nc.vector.tensor_copy(vct[:, c * CT:(c + 1) * CT], vps[i][:])
            for tt in range(chunks[0] * 4, (chunks[-1] + 1) * 4):
                tps = pss.tile([P, P], BF16, tag="ss", name="tps")
                nc.tensor.transpose(tps[:], vct[:, tt * P:(tt + 1) * P],
                                    ident[:])
                nc.vector.tensor_copy(vsb[tt][:], tps[:])

        qp0 = psp.tile([P, CT], F32, tag="pp", name="qp0")
        gp0 = psp.tile([P, CT], F32, tag="pp", name="gp0")
        proj_pass(0, (0, 1), (qp0, gp0))
        q_sb0 = sbq.tile([P, CT], BF16, tag="q_sb", bufs=4, name="q_sb0")
        nc.vector.tensor_copy(q_sb0[:], qp0[:])
        g_sb0 = sbq.tile([P, CT], BF16, tag="g_sb", bufs=5, name="g_sb0")
        nc.vector.tensor_copy(g_sb0[:], gp0[:])
        pre_pairs = {0: (q_sb0, g_sb0)}

        # ---- phase 1: per tq-chunk: q/gate proj, attention ----
        # o_proj for chunk c-1 is emitted after chunk c's norm chains so the
        # PE has dense work while the chains' DVE/ACT latency drains.
        def _o_proj(oc, og, heads=tuple(range(NHL))):
            ocs = slice(oc * CT, (oc + 1) * CT)
            # the final o_proj has the PSUM to itself: rotate all four tag
            # groups so psum-free never gates the accumulation chains
            pools4 = [(psp, "pp"), (pss, "ss"), (psa, "aa"), (psr, "row")]
            for dt in range(DT):
                ds_ = slice(dt * P, (dt + 1) * P)
                pl, tg = pools4[dt % 4]
                pso = pl.tile([P, CT], F32, tag=tg, name="pso")
                for i2, ct4 in enumerate(heads):
                    nc.tensor.matmul(pso[:], wo[ct4][:, ds_], og[ct4][:],
                                     start=(i2 == 0),
                                     stop=(i2 == len(heads) - 1))
                osb = sbw.tile([P, CT], BF16, tag="osb", bufs=4, name="osb")
                if dt % 2 == 0:
                    nc.vector.tensor_copy(osb[:], pso[:])
                else:
                    nc.scalar.copy(osb[:], pso[:])
                rings[dt % 3].dma_start(out_d[ds_, ocs], osb[:])

        prev_gated = None
        for c in range(CH):
            cs = slice(c * CT, (c + 1) * CT)
            q_sbs = []
            g_sbs = []
            sigs = []
            qrots = []

            chain_absr = []

            def _q_chain(h, c=None, cs=None, q_sbs=None, qrots=None):
                qrot = sbw.tile([P, CT], BF16, tag="qrot", bufs=4,
                                name="qrot")
                a = _norm_rope(nc, (sbw, sbr), psr, pss, ones_col[:],
                               ones_row[:], eps_t[:], q_sbs[h][:], qw[:],
                               cost[:, cs], sinpm[:, cs], qrot[:], CT)
                qrots.append(qrot)
                chain_absr.append(a)

            def _proj(w_t, h, dst_tag, bufs_):
                ps = psp.tile([P, CT], F32, tag="pp")
                for d in range(DT):
                    nc.tensor.matmul(ps[:], w_t[h][d][:], hidsl(d, c),
                                     start=(d == 0), stop=(d == DT - 1))
                sb = sbq.tile([P, CT], BF16, tag=dst_tag, bufs=bufs_)
                nc.vector.tensor_copy(sb[:], ps[:])
                return sb

            for h in range(NHL):
                if c == 0:
                    break
                if h == 0 and c in pre_pairs:
                    q_sbs.append(pre_pairs[c][0])
                    g_sbs.append(pre_pairs[c][1])
                    chain_absr.append(_norm_rope(
                        nc, (sbw, sbr), psr, pss, ones_col[:],
                        ones_row[:], eps_t[:], kbfs[c][:], kw[:],
                        cost[:, cs], sinpm[:, cs], krot[:, cs], CT))
                    continue
                ps = psp.tile([P, CT], F32, tag="pp")
                for d in range(DT):
                    nc.tensor.matmul(ps[:], wqqt[h][d][:], hidsl(d, c),
                                     start=(d == 0), stop=(d == DT - 1))
                q_sb = sbq.tile([P, CT], BF16, tag="q_sb", bufs=4)
                nc.vector.tensor_copy(q_sb[:], ps[:])
                q_sbs.append(q_sb)
                ps2 = psp.tile([P, CT], F32, tag="pp")
                for d in range(DT):
                    nc.tensor.matmul(ps2[:], wqgt[h][d][:], hidsl(d, c),
                                     start=(d == 0), stop=(d == DT - 1))
                g_sb = sbq.tile([P, CT], BF16, tag="g_sb", bufs=5)
                nc.vector.tensor_copy(g_sb[:], ps2[:])
                g_sbs.append(g_sb)
                # sandwich a norm/rope chain after each proj pair so the
                # chain's DVE/ACT latency hides behind the next pair's mms
                if h == 0:
                    chain_absr.append(_norm_rope(
                        nc, (sbw, sbr), psr, pss, ones_col[:],
                        ones_row[:], eps_t[:], kbfs[c][:], kw[:],
                        cost[:, cs], sinpm[:, cs], krot[:, cs], CT))
                else:
                    _q_chain(h - 1, c=c, cs=cs, q_sbs=q_sbs, qrots=qrots)
            if c != 0:
                _q_chain(NHL - 1, c=c, cs=cs, q_sbs=q_sbs, qrots=qrots)

            def _emit_sigs():
                for h in range(NHL):
                    # e^-g; the sigmoid is folded into the softmax
                    # denominator: attn*sig(g)/den == attn/((1+e^-g)*den)
                    sig = sbq.tile([P, CT], BF16, tag="sig", bufs=4,
                                   name="sig")
                    si = nc.scalar.activation(sig[:], g_sbs[h][:], AF.Exp,
                                              scale=-1.0)
                    # keep the chunk's absr chain ops contiguous on ACT:
                    # each exp<->absr interleave costs a ~1.3us table load
                    bass._add_dep_helper(si.ins, chain_absr[-1].ins,
                                         sync=False,
                                         reason="group exps after absr")
                    sigs.append(sig)
            gated = []
            nm = 4 * c + 4
            # Filler work drip-fed between attention m-steps keeps the PE
            # dense while ACT runs the exps: o_proj(c-1) tiles, and for
            # chunk 0 (which has no prior o_proj) the next chunk's first
            # projection pair.
            fillers = []
            if prev_gated is not None:
                ocs = slice((c - 1) * CT, c * CT)

                def _mk_oproj(dt, ocs=ocs, og=prev_gated):
                    def run():
                        ds_ = slice(dt * P, (dt + 1) * P)
                        pso = psp.tile([P, CT], F32, tag="pp", name="pso")
                        for ct4 in range(NHL):
                            nc.tensor.matmul(pso[:], wo[ct4][:, ds_],
                                             og[ct4][:], start=(ct4 == 0),
                                             stop=(ct4 == NHL - 1))
                        osb = sbw.tile([P, CT], BF16, tag="osb", bufs=4,
                                       name="osb")
                        if dt % 2 == 0:
                            nc.vector.tensor_copy(osb[:], pso[:])
                        else:
                            nc.scalar.copy(osb[:], pso[:])
                        nc.sync.dma_start(out_d[ds_, ocs], osb[:])
                    return run
                fillers += [_mk_oproj(dt) for dt in range(DT)]
            fill = {"i": 0}
            n_steps = 2 * nm + 6

            def _fill_tick(step):
                due = min(len(fillers),
                          len(fillers) * (step + 1) // n_steps + 1)
                while fill["i"] < due:
                    fillers[fill["i"]]()
                    fill["i"] += 1

            step_no = [0]
            pair_state = {}

            def _attn_mloop(hp):
                pair = (hp, hp + 1)
                attns = {h: psa.tile([P, CT], F32, tag="aa",
                                     name=f"attn{h}") for h in pair}
                esums = {}
                for m in range(nm):
                    ks = slice(m * P, (m + 1) * P)
                    r = m - 4 * c
                    lo = P * r if r > 0 else 0
                    ns = slice(lo, CT)
                    Es = {}
                    for h in pair:
                        sps = pss.tile([P, CT], F32, tag="ss", name="sps")
                        nc.tensor.matmul(sps[:, ns], krot[:, ks],
                                         qrots[h][:, ns],
                                         start=True, stop=True)
                        E = sbw.tile([P, CT], BF16, tag="E", name="E",
                                     bufs=6)
                        nc.scalar.activation(E[:, ns], sps[:, ns], AF.Exp,
                                             scale=SCALE)
                        if r >= 0:
                            nc.gpsimd.tensor_tensor(
                                E[:, lo:lo + P], E[:, lo:lo + P],
                                masks[:, 0:P], op=ALU.mult)
                        Es[h] = E
                    _fill_tick(step_no[0])
                    step_no[0] += 1
                    for h in pair:
                        # running softmax denominator in SBUF: out-of-place
                        # ping-pong adds keep the DVE in 2x packed mode; the
                        # diagonal (partial-width) steps accumulate in place
                        # to preserve the untouched columns
                        if m == 0 and c == 0:
                            # chunk 0's later steps accumulate in place, so
                            # the running sum must not pin an E-ring slot
                            ne = sbw.tile([P, CT], BF16, tag="esum",
                                          bufs=4, name="esum")
                            nc.vector.tensor_copy(ne[:], Es[h][:])
                            esums[h] = ne
                        elif m == 0:
                            esums[h] = Es[h]
                        elif r > 0:
                            nc.vector.tensor_tensor(
                                esums[h][:, ns], esums[h][:, ns],
                                Es[h][:, ns], op=ALU.add)
                        else:
                            ne = sbw.tile([P, CT], BF16, tag="esum",
                                          bufs=4, name="esum")
                            nc.vector.tensor_tensor(ne[:], esums[h][:],
                                                    Es[h][:], op=ALU.add)
                            esums[h] = ne
                        nc.tensor.matmul(attns[h][:, ns], vsb[m][:],
                                         Es[h][:, ns], start=(m == 0),
                                         stop=(m == nm - 1))
                _fill_tick(step_no[0])
                step_no[0] += 1
                pair_state[hp] = (attns, esums)

            def _attn_drain(hp):
                pair = (hp, hp + 1)
                attns, esums = pair_state.pop(hp)
                dpss = {}
                for h in pair:
                    dps = psr.tile([1, CT], F32, tag="row", name="dps")
                    nc.tensor.matmul(dps[:], ones_col[:], esums[h][:],
                                     start=True, stop=True)
                    dpss[h] = dps
                _fill_tick(step_no[0])
                step_no[0] += 1
                for h in pair:
                    dn = sbr.tile([1, CT], BF16, tag="rowtmp", name="dn")
                    nc.vector.tensor_copy(dn[:], dpss[h][:])
                    rb = psr.tile([P, CT], F32, tag="row", name="rb")
                    nc.tensor.matmul(rb[:], ones_row[:], dn[:],
                                     start=True, stop=True)
                    gd = sbw.tile([P, CT], F32, tag="rcb", name="gd",
                                  bufs=2)
                    nc.vector.scalar_tensor_tensor(gd[:], sigs[h][:], 1.0,
                                                   rb[:], op0=ALU.add,
                                                   op1=ALU.mult)
                    rcb = sbw.tile([P, CT], F32, tag="rcb", name="rcb",
                                   bufs=2)
                    nc.vector.reciprocal_approx_fast(out=rcb[:], in_=gd[:])
                    g = sbq.tile([P, CT], BF16, tag="gated", bufs=8)
                    nc.vector.tensor_tensor(g[:], attns[h][:], rcb[:],
                                            op=ALU.mult)
                    gated.append(g)
                _fill_tick(step_no[0])
                step_no[0] += 1
                if c == CH - 1 and hp == 0:
                    # heads 0-1's half of this chunk's o_proj can run as
                    # fillers during pair (2,3); host adds out2 to out_t
                    g01 = list(gated)

                    def _mk_ohalf(dt, g01=g01):
                        def run():
                            ds_ = slice(dt * P, (dt + 1) * P)
                            pso = psp.tile([P, CT], F32, tag="pp",
                                           name="psoh")
                            for i2 in range(2):
                                nc.tensor.matmul(pso[:], wo[i2][:, ds_],
                                                 g01[i2][:], start=(i2 == 0),
                                                 stop=(i2 == 1))
                            osb = sbw.tile([P, CT], BF16, tag="osb", bufs=4,
                                           name="osbh")
                            if dt % 2 == 0:
                                nc.vector.tensor_copy(osb[:], pso[:])
                            else:
                                nc.scalar.copy(osb[:], pso[:])
                            nc.sync.dma_start(out2_d[ds_, :], osb[:])
                        return run
                    fillers.extend(_mk_ohalf(dt) for dt in range(DT))

            if c == 0:
                # wqg trails the DMA stripe: all q-projections + chains
                # first (wqq rides the stripe), then pair (0,1) attention
                # runs while the wqg stream lands, then gate projections
                q_sbs.append(pre_pairs[0][0])
                g_sbs.append(pre_pairs[0][1])
                q_sbs.append(_proj(wqqt, 1, "q_sb", 4))
                chain_absr.append(_norm_rope(
                    nc, (sbw, sbr), psr, pss, ones_col[:], ones_row[:],
                    eps_t[:], kbfs[0][:], kw[:], cost[:, cs], sinpm[:, cs],
                    krot[:, cs], CT))
                q_sbs.append(_proj(wqqt, 2, "q_sb", 4))
                _q_chain(0, c=c, cs=cs, q_sbs=q_sbs, qrots=qrots)
                q_sbs.append(_proj(wqqt, 3, "q_sb", 4))
                _q_chain(1, c=c, cs=cs, q_sbs=q_sbs, qrots=qrots)
                _q_chain(2, c=c, cs=cs, q_sbs=q_sbs, qrots=qrots)
                _q_chain(3, c=c, cs=cs, q_sbs=q_sbs, qrots=qrots)
                _attn_mloop(0)
                # phase 0 pass B: k/v for chunks 2-3 from hid half 1 --
                # wqg-independent PE work that covers the gate-weight DMA
                proj_pass(1, (2, 3), None)
                g_sbs.append(_proj(wqgt, 1, "g_sb", 5))
                g_sbs.append(_proj(wqgt, 2, "g_sb", 5))
                g_sbs.append(_proj(wqgt, 3, "g_sb", 5))
                # chunk-1 first-pair prefetch, drip-fed as pair-(2,3) filler
                qp1 = psp.tile([P, CT], F32, tag="pp", name="qp1")
                gp1 = psp.tile([P, CT], F32, tag="pp", name="gp1")

                def _mk_proj(ps_t, w_t, dlist):
                    def run():
                        for d in dlist:
                            nc.tensor.matmul(
                                ps_t[:], w_t[0][d][:], hidsl(d, 1),
                                start=(d == 0), stop=(d == DT - 1))
                    return run
                for d0 in range(0, DT, 4):
                    fillers.append(_mk_proj(qp1, wqqt, range(d0, d0 + 4)))
                for d0 in range(0, DT, 4):
                    fillers.append(_mk_proj(gp1, wqgt, range(d0, d0 + 4)))
                _emit_sigs()
                _attn_drain(0)
                _attn_mloop(2)
                _attn_drain(2)
            else:
                _emit_sigs()
                for hp in (0, 2):
                    _attn_mloop(hp)
                    _attn_drain(hp)

            while fill["i"] < len(fillers):
                fillers[fill["i"]]()
                fill["i"] += 1
            if c == 0:
                q_sb1 = sbq.tile([P, CT], BF16, tag="q_sb", bufs=4,
                                 name="q_sb1")
                nc.vector.tensor_copy(q_sb1[:], qp1[:])
                g_sb1 = sbq.tile([P, CT], BF16, tag="g_sb", bufs=5,
                                 name="g_sb1")
                nc.vector.tensor_copy(g_sb1[:], gp1[:])
                pre_pairs[1] = (q_sb1, g_sb1)
            prev_gated = gated
        _o_proj(CH - 1, prev_gated, heads=(2, 3))
    nc.compile()
    return nc


def make_in_maps(hidden, cos, sin, wq, wk, wv, wo, q_norm_w, k_norm_w):
    """Build the 8 per-core input maps (host-side sharding + layout prep)."""
    i_idx = np.arange(P)[:, None]
    j_idx = np.arange(P)[None, :]
    masks = (j_idx >= i_idx).astype(BF)
    in_maps = []
    for core in range(N_CORES):
        b, g = core // NKV, core % NKV
        heads = range(NHL * g, NHL * g + NHL)
        sin_t = sin[b].T.copy()
        sin_t[:HD // 2] = -sin_t[:HD // 2]
        in_maps.append({
            "hid": np.ascontiguousarray(hidden[b].T).astype(BF),
            "wqq": np.concatenate(
                [wq[:, h * 2 * HD: h * 2 * HD + HD] for h in heads], 0
            ).astype(BF),
            "wqg": np.concatenate(
                [wq[:, h * 2 * HD + HD: (h + 1) * 2 * HD] for h in heads], 0
            ).astype(BF),
            "wk": np.ascontiguousarray(wk[:, g * HD:(g + 1) * HD]).astype(BF),
            "wv": np.ascontiguousarray(wv[:, g * HD:(g + 1) * HD]).astype(BF),
            "wo": np.ascontiguousarray(
                wo[NHL * HD * g: NHL * HD * (g + 1), :]).astype(BF),
            "cost": np.ascontiguousarray(cos[b].T).astype(BF),
            "sinpm": np.ascontiguousarray(sin_t).astype(BF),
            "qw": np.ascontiguousarray(q_norm_w[:, None]).astype(np.float32),
            "kw": np.ascontiguousarray(k_norm_w[:, None]).astype(np.float32),
            "masks": np.ascontiguousarray(masks),
        })
    return in_maps


def _install_ntff_hook():
    """Inject antenv.axon_hooks with a ctypes NTFF profile hook.

    The container's antenv package lacks axon_hooks, so bass_utils'
    trace=True path can't find the hook. Replicates the boot script's
    _ntff_profile_via_ctypes against libaxon_pjrt.so.
    """
    import contextlib
    import ctypes
    import types

    if "antenv.axon_hooks" in sys.modules:
        return
    lib = None
    for so_path in ("/opt/axon/libaxon_pjrt.so",
                    "/root/.axon_site/axon/libaxon_pjrt.so"):
        try:
            lib = ctypes.CDLL(so_path)
            break
        except OSError:
            continue
    if lib is None:
        return
    if not hasattr(lib, "axon_start_nrt_profile"):
        return
    lib.axon_start_nrt_profile.argtypes = [ctypes.POINTER(ctypes.c_int64),
                                           ctypes.c_size_t]
    lib.axon_start_nrt_profile.restype = ctypes.c_int64
    lib.axon_stop_nrt_profile.argtypes = [ctypes.c_char_p]
    lib.axon_stop_nrt_profile.restype = ctypes.c_int64

    @contextlib.contextmanager
    def _hook(output_dir, device_ids):
        import jax

        jax.devices()
        if device_ids:
            ids = (ctypes.c_int64 * len(device_ids))(*device_ids)
            rc = lib.axon_start_nrt_profile(ids, len(device_ids))
        else:
            rc = lib.axon_start_nrt_profile(None, 0)
        if rc != 0:
            raise RuntimeError(f"axon_start_nrt_profile rc={rc}")
        try:
            yield
        finally:
            n = lib.axon_stop_nrt_profile(str(output_dir).encode())
            print(f"profile: {n} file(s) written to {output_dir}",
                  file=sys.stderr)

    m = types.ModuleType("antenv.axon_hooks")
    m.get_axon_ntff_profile_hook = lambda: _hook
    m.set_axon_ntff_profile_hook = lambda h: None
    sys.modules["antenv.axon_hooks"] = m


_NC_CACHE = None


def _get_nc():
    global _NC_CACHE
    if _NC_CACHE is None:
        _NC_CACHE = build_nc()
    return _NC_CACHE


def kernel(hidden_BTD, cos_BTK, sin_BTK, wq, wk, wv, wo, q_norm_w, k_norm_w,
           segment_ids_BT=None, position_ids_BT=None, **_unused):
    from concourse.bass_utils import run_bass_kernel_spmd

    in_maps = make_in_maps(
        np.asarray(hidden_BTD, np.float32), np.asarray(cos_BTK, np.float32),
        np.asarray(sin_BTK, np.float32), np.asarray(wq, np.float32),
        np.asarray(wk, np.float32), np.asarray(wv, np.float32),
        np.asarray(wo, np.float32), np.asarray(q_norm_w, np.float32),
        np.asarray(k_norm_w, np.float32))
    nc = _get_nc()
    trace = bool(int(os.environ.get("BASS_KERNEL_TRACE", "0")))
    if trace:
        _install_ntff_hook()
    res = run_bass_kernel_spmd(nc, in_maps, core_ids=list(range(N_CORES)),
                               trace=trace)
    out = np.zeros((B, T, D), np.float32)
    for core in range(N_CORES):
        out[core // NKV] += res.results[core]["out_t"].astype(np.float32).T
        out[core // NKV, (CH - 1) * CT:] += \
            res.results[core]["out2"].astype(np.float32).T
    kernel.last_exec_time_ns = res.exec_time_ns
    kernel.last_results = res
    return out


kernel.last_exec_time_ns = None
kernel.last_results = None


# revision 34
# speedup vs baseline: 1.2300x; 1.2300x over previous
"""Trainium2 Bass kernel for gated GQA attention (nn_Attention_6476810683032).

Sharding: 8 cores = 2 (batch DP) x 4 (head-group TP).
Core c handles batch b=c//4, head group g=c%4 (q-heads 4g..4g+3, kv-head g).
Each core computes a partial o_proj output [D, T] (its 4 heads' contribution,
transposed layout); the host sums the 4 partials per batch and transposes.

On-device per core (all matmuls bf16 with fp32 PSUM accumulation):
  - projections from host-pre-transposed hidden_t [D, T] (channel-major
    outputs for q/gate/k, token-major for v) -- no on-device transposes
  - RMS norm via ones-matmul partition reduction + K=1 broadcast matmul
  - RoPE via partition-offset elementwise ops with a pre-signed sin table
  - causal attention in transposed-score form: S_T[tk,tq] = k_rot.T@q_rot,
    exp without max subtraction (logits bounded by the RMS norms)
  - softmax denominator: E tiles accumulate in SBUF on the DVE (bf16
    ping-pong adds stay in the 2x packed mode; partial-width diagonal
    steps add in place), then ONE ones-matmul per (chunk, head) -- saves
    ~60k PE columns (~30us) vs a per-m-step ones-matmul
  - sigmoid folded into the normalization: attn*sig(g)/den ==
    attn/((1+e^-g)*den); e^-g shares the exp ACT table, so the whole
    kernel uses Exp + Abs_reciprocal_sqrt only (9 table loads; eg exps
    are dep-ordered after the chunk's absr chain to prevent interleave)
  - partial o_proj: out_T[dout,t] = wo_slice.T @ gated (bf16 partials,
    summed in f32 on the host); the last chunk's heads-0/1 half goes to
    out2 as PE filler during the pair-(2,3) m-loop

Scheduling notes (PE p-state drops to 1.2GHz after any bubble and takes
~3us of continuous execution to return to 2.4GHz, so PE density is worth
more than engine-local optima; the chip also thermally throttles ~18%
run-to-run -- compare cold first-exec numbers or LDWEIGHTS-normalized):
  - hid loads in half-T tiles; pass A (k/v chunks 0-1 + head-0 q/gate)
    paces with the half-0 stripe; wqq/wqg are head-major 32KB tiles so
    each head's projection starts as its 0.5MB block lands
  - chunk 0 is fully reordered around the DMA stream: q-projections +
    all chains (wqq in-stripe), pair-(0,1) m-loop, pass B (k/v chunks
    2-3 from half 1), gate projections (wqg lands last), gating drains
  - input DMAs issue in first-use order, striped across the three rings
    (sync/scalar HWDGE + gpsimd SWDGE) balanced by bytes
  - o_proj of chunk c-1 + chunk-1 prefetch + last-chunk half o_proj are
    drip-fed between m-steps and into the drains as PE filler
  - final o_proj drains through 4 osb buffers, casts split DVE/ACT,
    writes striped across all three DMA rings
Measured: 419us baseline -> 376us (cold-clock best; ~81% PE occupancy).
"""

import os
import sys
from contextlib import ExitStack

import numpy as np

sys.path.insert(0, "/opt/trn_rl_repo")

import ml_dtypes  # noqa: E402

import concourse.bass as bass  # noqa: E402
import concourse.mybir as mybir  # noqa: E402
import concourse.tile as tile  # noqa: E402
from concourse import bacc  # noqa: E402
from concourse import masks as masks_mod  # noqa: E402

F32 = mybir.dt.float32
BF16 = mybir.dt.bfloat16
AF = mybir.ActivationFunctionType
ALU = mybir.AluOpType
BF = ml_dtypes.bfloat16

P = 128
B, T, D = 2, 2048, 2048
NH, NKV, HD = 16, 4, 128
NHL = NH // NKV          # local q heads per core (4)
CH = 4                   # tq chunks
CT = T // CH             # 512 tokens per chunk
TH = T // 2              # half-T (hid DMA tile width)
DT = D // P              # 16 contraction tiles
KT = T // P              # 16 tk tiles
EPS = 1e-6
SCALE = HD ** -0.5
N_CORES = 8


def _norm_rope(nc, pools, psr, pss, ones_col, ones_row, eps_t, x_bf, w_ap,
               cos_sl, sin_sl, out_ap, n):
    """RMS-norm (over partitions) + RoPE on a [128, n] channel-major tile.

    x_bf: [128, n] bf16 SBUF (pre-norm channels-on-partitions tile)
    w_ap: [128, 1] f32 norm weight
    cos_sl/sin_sl: [128, n] bf16 (sin pre-signed: rows 0-63 negated)
    out_ap: [128, n] bf16 destination
    """
    sbw, sbr = pools
    xsq = sbw.tile([P, n], BF16, tag="tmpa", name="xsq")
    nc.vector.tensor_tensor(xsq[:], x_bf, x_bf, op=ALU.mult)
    ssq = psr.tile([1, n], F32, tag="row", name="ssq")
    nc.tensor.matmul(ssq[:], ones_col, xsq[:], start=True, stop=True)
    rsq = sbr.tile([1, n], BF16, tag="rsq", name="rsq")
    absr = nc.scalar.activation(rsq[:], ssq[:], AF.Abs_reciprocal_sqrt,
                                scale=1.0 / HD, bias=eps_t)
    rb = pss.tile([P, n], F32, tag="ss", name="rb")
    nc.tensor.matmul(rb[:], ones_row, rsq[:], start=True, stop=True)
    wr = sbw.tile([P, n], BF16, tag="tmpb", name="wr")
    nc.vector.tensor_scalar(wr[:], rb[:], w_ap, None, ALU.mult)
    xn = sbw.tile([P, n], BF16, tag="xn", name="xn")
    nc.vector.tensor_tensor(xn[:], x_bf, wr[:], op=ALU.mult)
    t1 = sbw.tile([P, n], BF16, tag="tmpb", name="t1")
    nc.vector.tensor_tensor(t1[:], xn[:], cos_sl, op=ALU.mult)
    h = HD // 2
    xs = sbw.tile([P, n], BF16, tag="tmpc", name="xs", bufs=2)
    nc.vector.tensor_copy(xs[0:h, :], xn[h:P, :])
    nc.vector.tensor_copy(xs[h:P, :], xn[0:h, :])
    t2 = sbw.tile([P, n], BF16, tag="tmpa", name="t2")
    nc.vector.tensor_tensor(t2[:], xs[:], sin_sl, op=ALU.mult)
    nc.vector.tensor_tensor(out_ap, t1[:], t2[:], op=ALU.add)
    return absr


def build_nc():
    nc = bacc.Bacc("TRN2", target_bir_lowering=False, debug=False,
                   num_devices=N_CORES)
    hid_d = nc.dram_tensor("hid", [D, T], BF16, kind="ExternalInput")
    wqq_d = nc.dram_tensor("wqq", [NHL * D, HD], BF16, kind="ExternalInput")
    wqg_d = nc.dram_tensor("wqg", [NHL * D, HD], BF16, kind="ExternalInput")
    wk_d = nc.dram_tensor("wk", [D, HD], BF16, kind="ExternalInput")
    wv_d = nc.dram_tensor("wv", [D, HD], BF16, kind="ExternalInput")
    wo_d = nc.dram_tensor("wo", [NHL * HD, D], BF16, kind="ExternalInput")
    cos_d = nc.dram_tensor("cost", [P, T], BF16, kind="ExternalInput")
    sin_d = nc.dram_tensor("sinpm", [P, T], BF16, kind="ExternalInput")
    qw_d = nc.dram_tensor("qw", [P, 1], F32, kind="ExternalInput")
    kw_d = nc.dram_tensor("kw", [P, 1], F32, kind="ExternalInput")
    mask_d = nc.dram_tensor("masks", [P, P], BF16, kind="ExternalInput")
    out_d = nc.dram_tensor("out_t", [D, T], BF16, kind="ExternalOutput")
    # heads 0-1's half of the last chunk's o_proj (host adds the two halves)
    out2_d = nc.dram_tensor("out2", [D, CT], BF16, kind="ExternalOutput")

    with tile.TileContext(nc) as tc, ExitStack() as ctx, \
            nc.allow_low_precision(reason="bf16 softmax temps validated by rel_err"):
        sbp = ctx.enter_context(tc.tile_pool(name="sbp", bufs=1))
        sbw = ctx.enter_context(tc.tile_pool(name="sbw", bufs=3))
        sbr = ctx.enter_context(tc.tile_pool(name="sbr", bufs=2))
        sbq = ctx.enter_context(tc.tile_pool(name="sbq", bufs=6))
        psp = ctx.enter_context(tc.tile_pool(name="psp", bufs=2, space="PSUM"))
        pss = ctx.enter_context(tc.tile_pool(name="pss", bufs=2, space="PSUM"))
        psa = ctx.enter_context(tc.tile_pool(name="psa", bufs=2, space="PSUM"))
        psr = ctx.enter_context(tc.tile_pool(name="psr", bufs=2, space="PSUM"))

        # ---- persistent tiles ----
        wkT = [sbp.tile([P, HD], BF16, tag=f"wk{i}", name=f"wk{i}")
               for i in range(DT)]
        wvT = [sbp.tile([P, HD], BF16, tag=f"wv{i}", name=f"wv{i}")
               for i in range(DT)]
        hidh = [[sbp.tile([P, TH], BF16, tag=f"hid{d}_{j}", name=f"hid{d}_{j}")
                 for j in range(2)] for d in range(DT)]
        wqqt = [[sbp.tile([P, HD], BF16, tag=f"wqq{h}_{d}",
                         name=f"wqq{h}_{d}") for d in range(DT)]
                for h in range(NHL)]
        wqgt = [[sbp.tile([P, HD], BF16, tag=f"wqg{h}_{d}",
                         name=f"wqg{h}_{d}") for d in range(DT)]
                for h in range(NHL)]
        wo = [sbp.tile([P, D], BF16, tag=f"wo{i}", name=f"wo{i}")
              for i in range(NHL)]
        cost = sbp.tile([P, T], BF16, tag="cost")
        sinpm = sbp.tile([P, T], BF16, tag="sinpm")
        masks = sbp.tile([P, P], BF16, tag="masks")
        qw = sbp.tile([P, 1], F32, tag="qw")
        kw = sbp.tile([P, 1], F32, tag="kw")

        def hidsl(d, c):
            j, o = divmod(c, 2)
            return hidh[d][j][:, o * CT:(o + 1) * CT]

        # ---- DMA issue: first-use order, striped across the 3 rings ----
        rings = [nc.sync, nc.scalar, nc.gpsimd]
        ring_bytes = [0, 0, 0]

        def load(dst, src):
            i = ring_bytes.index(min(ring_bytes))
            rings[i].dma_start(dst, src)
            ring_bytes[i] += dst.size() * 2

        nc.gpsimd.dma_start(qw[:], qw_d[:, :])
        nc.gpsimd.dma_start(kw[:], kw_d[:, :])
        nc.gpsimd.dma_start(masks[:], mask_d[:, :])
        ring_bytes[2] += 64 * 1024
        for d in range(DT):
            ds_ = slice(d * P, (d + 1) * P)
            load(wkT[d][:], wk_d[ds_, :])
            load(wvT[d][:], wv_d[ds_, :])
            load(hidh[d][0][:], hid_d[ds_, 0:TH])
            # head-0 q/gate weights ride the stripe (pass A needs them)
            load(wqqt[0][d][:], wqq_d[d * P:(d + 1) * P, :])
            load(wqgt[0][d][:], wqg_d[d * P:(d + 1) * P, :])
            # rope-table chunk slices just ahead of their chains' first use
            if d == 3:
                load(cost[:, 0:CT], cos_d[:, 0:CT])
                load(sinpm[:, 0:CT], sin_d[:, 0:CT])
            if d == 11:
                load(cost[:, CT:TH], cos_d[:, CT:TH])
                load(sinpm[:, CT:TH], sin_d[:, CT:TH])
        for h in range(1, NHL):
            for d in range(DT):
                ds_ = slice((h * D + d * P), (h * D + (d + 1) * P))
                load(wqqt[h][d][:], wqq_d[ds_, :])
        for d in range(DT):
            ds_ = slice(d * P, (d + 1) * P)
            load(hidh[d][1][:], hid_d[ds_, TH:T])
        for h in range(1, NHL):
            for d in range(DT):
                ds_ = slice((h * D + d * P), (h * D + (d + 1) * P))
                load(wqgt[h][d][:], wqg_d[ds_, :])
        load(cost[:, TH:T], cos_d[:, TH:T])
        load(sinpm[:, TH:T], sin_d[:, TH:T])
        for i in range(NHL):
            load(wo[i][:], wo_d[i * P:(i + 1) * P, :])

        ones_col = sbp.tile([P, 1], BF16, tag="ones_col")
        nc.vector.memset(ones_col[:], 1.0)
        ones_row = sbp.tile([1, P], BF16, tag="ones_row")
        nc.vector.memset(ones_row[:], 1.0)
        eps_t = sbp.tile([1, 1], F32, tag="eps_t")
        nc.vector.memset(eps_t[:], EPS)
        ident = sbp.tile([P, P], BF16, tag="ident")
        masks_mod.make_identity(nc, ident[:])
        krot = sbp.tile([P, T], BF16, tag="krot")
        vct = sbp.tile([P, T], BF16, tag="vct")
        vsb = []
        for i in range(KT):
            vsb.append(sbp.tile([P, HD], BF16, tag=f"v{i}", name=f"v{i}"))

        kbfs = []

        # ---- phase 0 pass A: k/v chunks 0-1 + chunk-0 q/gate, d-outer so
        # the PE paces with the hid half-0 DMA stream.
        def proj_pass(half, chunks, qg_psum):
            kps = [pss.tile([P, CT], F32, tag="ss", name=f"kps{c}")
                   for c in chunks]
            vps = [psr.tile([P, CT], F32, tag="row", name=f"vps{c}")
                   for c in chunks]
            for d in range(DT):
                st, sp = (d == 0), (d == DT - 1)
                for i, c in enumerate(chunks):
                    nc.tensor.matmul(kps[i][:], wkT[d][:, :], hidsl(d, c),
                                     start=st, stop=sp)
                    nc.tensor.matmul(vps[i][:], wvT[d][:, :], hidsl(d, c),
                                     start=st, stop=sp)
                if qg_psum is not None:
                    nc.tensor.matmul(qg_psum[0][:], wqqt[0][d][:],
                                     hidsl(d, 0), start=st, stop=sp)
                    nc.tensor.matmul(qg_psum[1][:], wqgt[0][d][:],
                                     hidsl(d, 0), start=st, stop=sp)
            for i, c in enumerate(chunks):
                kbf = sbw.tile([P, CT], BF16, tag="kbf", name="kbf", bufs=4)
                nc.vector.tensor_copy(kbf[:], kps[i][:])
                kbfs.append(kbf)
                nc.vector.tensor_copy(vct[:, c * CT:(c + 1) * CT], vps[i][:])
            for tt in range(chunks[0] * 4, (chunks[-1] + 1) * 4):
                tps = pss.tile([P, P], BF16, tag="ss", name="tps")
                nc.tensor.transpose(tps[:], vct[:, tt * P:(tt + 1) * P],
                                    ident[:])
                nc.vector.tensor_copy(vsb[tt][:], tps[:])

        qp0 = psp.tile([P, CT], F32, tag="pp", name="qp0")
        gp0 = psp.tile([P, CT], F32, tag="pp", name="gp0")
        proj_pass(0, (0, 1), (qp0, gp0))
        q_sb0 = sbq.tile([P, CT], BF16, tag="q_sb", bufs=4, name="q_sb0")
        nc.vector.tensor_copy(q_sb0[:], qp0[:])
        g_sb0 = sbq.tile([P, CT], BF16, tag="g_sb", bufs=5, name="g_sb0")
        nc.vector.tensor_copy(g_sb0[:], gp0[:])
        pre_pairs = {0: (q_sb0, g_sb0)}

        # ---- phase 1: per tq-chunk: q/gate proj, attention ----
        # o_proj for chunk c-1 is emitted after chunk c's norm chains so the
        # PE has dense work while the chains' DVE/ACT latency drains.
        def _o_proj(oc, og, heads=tuple(range(NHL))):
            ocs = slice(oc * CT, (oc + 1) * CT)
            # the final o_proj has the PSUM to itself: rotate all four tag
            # groups so psum-free never gates the accumulation chains
            pools4 = [(psp, "pp"), (pss, "ss"), (psa, "aa"), (psr, "row")]
            for dt in range(DT):
                ds_ = slice(dt * P, (dt + 1) * P)
                pl, tg = pools4[dt % 4]
                pso = pl.tile([P, CT], F32, tag=tg, name="pso")
                for i2, ct4 in enumerate(heads):
                    nc.tensor.matmul(pso[:], wo[ct4][:, ds_], og[ct4][:],
                                     start=(i2 == 0),
                                     stop=(i2 == len(heads) - 1))
                osb = sbw.tile([P, CT], BF16, tag="osb", bufs=4, name="osb")
                if dt % 2 == 0:
                    nc.vector.tensor_copy(osb[:], pso[:])
                else:
                    nc.scalar.copy(osb[:], pso[:])
                rings[dt % 3].dma_start(out_d[ds_, ocs], osb[:])

        prev_gated = None
        for c in range(CH):
            cs = slice(c * CT, (c + 1) * CT)
            q_sbs = []
            g_sbs = []
            sigs = []
            qrots = []

            chain_absr = []

            def _q_chain(h, c=None, cs=None, q_sbs=None, qrots=None):
                qrot = sbw.tile([P, CT], BF16, tag="qrot", bufs=4,
                                name="qrot")
                a = _norm_rope(nc, (sbw, sbr), psr, pss, ones_col[:],
                               ones_row[:], eps_t[:], q_sbs[h][:], qw[:],
                               cost[:, cs], sinpm[:, cs], qrot[:], CT)
                qrots.append(qrot)
                chain_absr.append(a)

            def _proj(w_t, h, dst_tag, bufs_):
                ps = psp.tile([P, CT], F32, tag="pp")
                for d in range(DT):
                    nc.tensor.matmul(ps[:], w_t[h][d][:], hidsl(d, c),
                                     start=(d == 0), stop=(d == DT - 1))
                sb = sbq.tile([P, CT], BF16, tag=dst_tag, bufs=bufs_)
                nc.vector.tensor_copy(sb[:], ps[:])
                return sb

            for h in range(NHL):
                if c == 0:
                    break
                if h == 0 and c in pre_pairs:
                    q_sbs.append(pre_pairs[c][0])
                    g_sbs.append(pre_pairs[c][1])
                    chain_absr.append(_norm_rope(
                        nc, (sbw, sbr), psr, pss, ones_col[:],
                        ones_row[:], eps_t[:], kbfs[c][:], kw[:],
                        cost[:, cs], sinpm[:, cs], krot[:, cs], CT))
                    continue
                ps = psp.tile([P, CT], F32, tag="pp")
                for d in range(DT):
                    nc.tensor.matmul(ps[:], wqqt[h][d][:], hidsl(d, c),
                                     start=(d == 0), stop=(d == DT - 1))
                q_sb = sbq.tile([P, CT], BF16, tag="q_sb", bufs=4)
                nc.vector.tensor_copy(q_sb[:], ps[:])
                q_sbs.append(q_sb)
                ps2 = psp.tile([P, CT], F32, tag="pp")
                for d in range(DT):
                    nc.tensor.matmul(ps2[:], wqgt[h][d][:], hidsl(d, c),
                                     start=(d == 0), stop=(d == DT - 1))
                g_sb = sbq.tile([P, CT], BF16, tag="g_sb", bufs=5)
                nc.vector.tensor_copy(g_sb[:], ps2[:])
                g_sbs.append(g_sb)
                # sandwich a norm/rope chain after each proj pair so the
                # chain's DVE/ACT latency hides behind the next pair's mms
                if h == 0:
                    chain_absr.append(_norm_rope(
                        nc, (sbw, sbr), psr, pss, ones_col[:],
                        ones_row[:], eps_t[:], kbfs[c][:], kw[:],
                        cost[:, cs], sinpm[:, cs], krot[:, cs], CT))
                else:
                    _q_chain(h - 1, c=c, cs=cs, q_sbs=q_sbs, qrots=qrots)
            if c != 0:
                _q_chain(NHL - 1, c=c, cs=cs, q_sbs=q_sbs, qrots=qrots)

            def _emit_sigs():
                for h in range(NHL):
                    # e^-g; the sigmoid is folded into the softmax
                    # denominator: attn*sig(g)/den == attn/((1+e^-g)*den)
                    sig = sbq.tile([P, CT], BF16, tag="sig", bufs=4,
                                   name="sig")
                    si = nc.scalar.activation(sig[:], g_sbs[h][:], AF.Exp,
                                              scale=-1.0)
                    # keep the chunk's absr chain ops contiguous on ACT:
                    # each exp<->absr interleave costs a ~1.3us table load
                    bass._add_dep_helper(si.ins, chain_absr[-1].ins,
                                         sync=False,
                                         reason="group exps after absr")
                    sigs.append(sig)
            gated = []
            nm = 4 * c + 4
            # Filler work drip-fed between attention m-steps keeps the PE
            # dense while ACT runs the exps: o_proj(c-1) tiles, and for
            # chunk 0 (which has no prior o_proj) the next chunk's first
            # projection pair.
            fillers = []
            if prev_gated is not None:
                ocs = slice((c - 1) * CT, c * CT)

                def _mk_oproj(dt, ocs=ocs, og=prev_gated):
                    def run():
                        ds_ = slice(dt * P, (dt + 1) * P)
                        pso = psp.tile([P, CT], F32, tag="pp", name="pso")
                        for ct4 in range(NHL):
                            nc.tensor.matmul(pso[:], wo[ct4][:, ds_],
                                             og[ct4][:], start=(ct4 == 0),
                                             stop=(ct4 == NHL - 1))
                        osb = sbw.tile([P, CT], BF16, tag="osb", bufs=4,
                                       name="osb")
                        nc.vector.tensor_copy(osb[:], pso[:])
                        nc.sync.dma_start(out_d[ds_, ocs], osb[:])
                    return run
                fillers += [_mk_oproj(dt) for dt in range(DT)]
            fill = {"i": 0}
            n_steps = 2 * nm + 6

            def _fill_tick(step):
                due = min(len(fillers),
                          len(fillers) * (step + 1) // n_steps + 1)
                while fill["i"] < due:
                    fillers[fill["i"]]()
                    fill["i"] += 1

            step_no = [0]
            pair_state = {}

            def _attn_mloop(hp):
                pair = (hp, hp + 1)
                attns = {h: psa.tile([P, CT], F32, tag="aa",
                                     name=f"attn{h}") for h in pair}
                esums = {}
                for m in range(nm):
                    ks = slice(m * P, (m + 1) * P)
                    r = m - 4 * c
                    lo = P * r if r > 0 else 0
                    ns = slice(lo, CT)
                    Es = {}
                    for h in pair:
                        sps = pss.tile([P, CT], F32, tag="ss", name="sps")
                        nc.tensor.matmul(sps[:, ns], krot[:, ks],
                                         qrots[h][:, ns],
                                         start=True, stop=True)
                        E = sbw.tile([P, CT], BF16, tag="E", name="E",
                                     bufs=6)
                        nc.scalar.activation(E[:, ns], sps[:, ns], AF.Exp,
                                             scale=SCALE)
                        if r >= 0:
                            nc.gpsimd.tensor_tensor(
                                E[:, lo:lo + P], E[:, lo:lo + P],
                                masks[:, 0:P], op=ALU.mult)
                        Es[h] = E
                    _fill_tick(step_no[0])
                    step_no[0] += 1
                    for h in pair:
                        # running softmax denominator in SBUF: out-of-place
                        # ping-pong adds keep the DVE in 2x packed mode; the
                        # diagonal (partial-width) steps accumulate in place
                        # to preserve the untouched columns
                        if m == 0 and c == 0:
                            # chunk 0's later steps accumulate in place, so
                            # the running sum must not pin an E-ring slot
                            ne = sbw.tile([P, CT], BF16, tag="esum",
                                          bufs=4, name="esum")
                            nc.vector.tensor_copy(ne[:], Es[h][:])
                            esums[h] = ne
                        elif m == 0:
                            esums[h] = Es[h]
                        elif r > 0:
                            nc.vector.tensor_tensor(
                                esums[h][:, ns], esums[h][:, ns],
                                Es[h][:, ns], op=ALU.add)
                        else:
                            ne = sbw.tile([P, CT], BF16, tag="esum",
                                          bufs=4, name="esum")
                            nc.vector.tensor_tensor(ne[:], esums[h][:],
                                                    Es[h][:], op=ALU.add)
                            esums[h] = ne
                        nc.tensor.matmul(attns[h][:, ns], vsb[m][:],
                                         Es[h][:, ns], start=(m == 0),
                                         stop=(m == nm - 1))
                _fill_tick(step_no[0])
                step_no[0] += 1
                pair_state[hp] = (attns, esums)

            def _attn_drain(hp):
                pair = (hp, hp + 1)
                attns, esums = pair_state.pop(hp)
                dpss = {}
                for h in pair:
                    dps = psr.tile([1, CT], F32, tag="row", name="dps")
                    nc.tensor.matmul(dps[:], ones_col[:], esums[h][:],
                                     start=True, stop=True)
                    dpss[h] = dps
                _fill_tick(step_no[0])
                step_no[0] += 1
                for h in pair:
                    dn = sbr.tile([1, CT], BF16, tag="rowtmp", name="dn")
                    nc.vector.tensor_copy(dn[:], dpss[h][:])
                    rb = psr.tile([P, CT], F32, tag="row", name="rb")
                    nc.tensor.matmul(rb[:], ones_row[:], dn[:],
                                     start=True, stop=True)
                    gd = sbw.tile([P, CT], F32, tag="rcb", name="gd",
                                  bufs=2)
                    nc.vector.scalar_tensor_tensor(gd[:], sigs[h][:], 1.0,
                                                   rb[:], op0=ALU.add,
                                                   op1=ALU.mult)
                    rcb = sbw.tile([P, CT], F32, tag="rcb", name="rcb",
                                   bufs=2)
                    nc.vector.reciprocal_approx_fast(out=rcb[:], in_=gd[:])
                    g = sbq.tile([P, CT], BF16, tag="gated", bufs=8)
                    nc.vector.tensor_tensor(g[:], attns[h][:], rcb[:],
                                            op=ALU.mult)
                    gated.append(g)
                _fill_tick(step_no[0])
                step_no[0] += 1
                if c == CH - 1 and hp == 0:
                    # heads 0-1's half of this chunk's o_proj can run as
                    # fillers during pair (2,3); host adds out2 to out_t
                    g01 = list(gated)

                    def _mk_ohalf(dt, g01=g01):
                        def run():
                            ds_ = slice(dt * P, (dt + 1) * P)
                            pso = psp.tile([P, CT], F32, tag="pp",
                                           name="psoh")
                            for i2 in range(2):
                                nc.tensor.matmul(pso[:], wo[i2][:, ds_],
                                                 g01[i2][:], start=(i2 == 0),
                                                 stop=(i2 == 1))
                            osb = sbw.tile([P, CT], BF16, tag="osb", bufs=4,
                                           name="osbh")
                            nc.vector.tensor_copy(osb[:], pso[:])
                            nc.sync.dma_start(out2_d[ds_, :], osb[:])
                        return run
                    fillers.extend(_mk_ohalf(dt) for dt in range(DT))

            if c == 0:
                # wqg trails the DMA stripe: all q-projections + chains
                # first (wqq rides the stripe), then pair (0,1) attention
                # runs while the wqg stream lands, then gate projections
                q_sbs.append(pre_pairs[0][0])
                g_sbs.append(pre_pairs[0][1])
                q_sbs.append(_proj(wqqt, 1, "q_sb", 4))
                chain_absr.append(_norm_rope(
                    nc, (sbw, sbr), psr, pss, ones_col[:], ones_row[:],
                    eps_t[:], kbfs[0][:], kw[:], cost[:, cs], sinpm[:, cs],
                    krot[:, cs], CT))
                q_sbs.append(_proj(wqqt, 2, "q_sb", 4))
                _q_chain(0, c=c, cs=cs, q_sbs=q_sbs, qrots=qrots)
                q_sbs.append(_proj(wqqt, 3, "q_sb", 4))
                _q_chain(1, c=c, cs=cs, q_sbs=q_sbs, qrots=qrots)
                _q_chain(2, c=c, cs=cs, q_sbs=q_sbs, qrots=qrots)
                _q_chain(3, c=c, cs=cs, q_sbs=q_sbs, qrots=qrots)
                _attn_mloop(0)
                # phase 0 pass B: k/v for chunks 2-3 from hid half 1 --
                # wqg-independent PE work that covers the gate-weight DMA
                proj_pass(1, (2, 3), None)
                g_sbs.append(_proj(wqgt, 1, "g_sb", 5))
                g_sbs.append(_proj(wqgt, 2, "g_sb", 5))
                g_sbs.append(_proj(wqgt, 3, "g_sb", 5))
                # chunk-1 first-pair prefetch, drip-fed as pair-(2,3) filler
                qp1 = psp.tile([P, CT], F32, tag="pp", name="qp1")
                gp1 = psp.tile([P, CT], F32, tag="pp", name="gp1")

                def _mk_proj(ps_t, w_t, dlist):
                    def run():
                        for d in dlist:
                            nc.tensor.matmul(
                                ps_t[:], w_t[0][d][:], hidsl(d, 1),
                                start=(d == 0), stop=(d == DT - 1))
                    return run
                for d0 in range(0, DT, 4):
                    fillers.append(_mk_proj(qp1, wqqt, range(d0, d0 + 4)))
                for d0 in range(0, DT, 4):
                    fillers.append(_mk_proj(gp1, wqgt, range(d0, d0 + 4)))
                _emit_sigs()
                _attn_drain(0)
                _attn_mloop(2)
                _attn_drain(2)
            else:
                _emit_sigs()
                for hp in (0, 2):
                    _attn_mloop(hp)
                    _attn_drain(hp)

            while fill["i"] < len(fillers):
                fillers[fill["i"]]()
                fill["i"] += 1
            if c == 0:
                q_sb1 = sbq.tile([P, CT], BF16, tag="q_sb", bufs=4,
                                 name="q_sb1")
                nc.vector.tensor_copy(q_sb1[:], qp1[:])
                g_sb1 = sbq.tile([P, CT], BF16, tag="g_sb", bufs=5,
                                 name="g_sb1")
                nc.vector.tensor_copy(g_sb1[:], gp1[:])
                pre_pairs[1] = (q_sb1, g_sb1)
            prev_gated = gated
        _o_proj(CH - 1, prev_gated, heads=(2, 3))
    nc.compile()
    return nc


def make_in_maps(hidden, cos, sin, wq, wk, wv, wo, q_norm_w, k_norm_w):
    """Build the 8 per-core input maps (host-side sharding + layout prep)."""
    i_idx = np.arange(P)[:, None]
    j_idx = np.arange(P)[None, :]
    masks = (j_idx >= i_idx).astype(BF)
    in_maps = []
    for core in range(N_CORES):
        b, g = core // NKV, core % NKV
        heads = range(NHL * g, NHL * g + NHL)
        sin_t = sin[b].T.copy()
        sin_t[:HD // 2] = -sin_t[:HD // 2]
        in_maps.append({
            "hid": np.ascontiguousarray(hidden[b].T).astype(BF),
            "wqq": np.concatenate(
                [wq[:, h * 2 * HD: h * 2 * HD + HD] for h in heads], 0
            ).astype(BF),
            "wqg": np.concatenate(
                [wq[:, h * 2 * HD + HD: (h + 1) * 2 * HD] for h in heads], 0
            ).astype(BF),
            "wk": np.ascontiguousarray(wk[:, g * HD:(g + 1) * HD]).astype(BF),
            "wv": np.ascontiguousarray(wv[:, g * HD:(g + 1) * HD]).astype(BF),
            "wo": np.ascontiguousarray(
                wo[NHL * HD * g: NHL * HD * (g + 1), :]).astype(BF),
            "cost": np.ascontiguousarray(cos[b].T).astype(BF),
            "sinpm": np.ascontiguousarray(sin_t).astype(BF),
            "qw": np.ascontiguousarray(q_norm_w[:, None]).astype(np.float32),
            "kw": np.ascontiguousarray(k_norm_w[:, None]).astype(np.float32),
            "masks": np.ascontiguousarray(masks),
        })
    return in_maps


def _install_ntff_hook():
    """Inject antenv.axon_hooks with a ctypes NTFF profile hook.

    The container's antenv package lacks axon_hooks, so bass_utils'
    trace=True path can't find the hook. Replicates the boot script's
    _ntff_profile_via_ctypes against libaxon_pjrt.so.
    """
    import contextlib
    import ctypes
    import types

    if "antenv.axon_hooks" in sys.modules:
        return
    lib = None
    for so_path in ("/opt/axon/libaxon_pjrt.so",
                    "/root/.axon_site/axon/libaxon_pjrt.so"):
        try:
            lib = ctypes.CDLL(so_path)
            break
        except OSError:
            continue
    if lib is None:
        return
    if not hasattr(lib, "axon_start_nrt_profile"):
        return
    lib.axon_start_nrt_profile.argtypes = [ctypes.POINTER(ctypes.c_int64),
                                           ctypes.c_size_t]
    lib.axon_start_nrt_profile.restype = ctypes.c_int64
    lib.axon_stop_nrt_profile.argtypes = [ctypes.c_char_p]
    lib.axon_stop_nrt_profile.restype = ctypes.c_int64

    @contextlib.contextmanager
    def _hook(output_dir, device_ids):
        import jax

        jax.devices()
        if device_ids:
            ids = (ctypes.c_int64 * len(device_ids))(*device_ids)
            rc = lib.axon_start_nrt_profile(ids, len(device_ids))
        else:
            rc = lib.axon_start_nrt_profile(None, 0)
        if rc != 0:
            raise RuntimeError(f"axon_start_nrt_profile rc={rc}")
        try:
            yield
        finally:
            n = lib.axon_stop_nrt_profile(str(output_dir).encode())
            print(f"profile: {n} file(s) written to {output_dir}",
                  file=sys.stderr)

    m = types.ModuleType("antenv.axon_hooks")
    m.get_axon_ntff_profile_hook = lambda: _hook
    m.set_axon_ntff_profile_hook = lambda h: None
    sys.modules["antenv.axon_hooks"] = m


_NC_CACHE = None


def _get_nc():
    global _NC_CACHE
    if _NC_CACHE is None:
        _NC_CACHE = build_nc()
    return _NC_CACHE


def kernel(hidden_BTD, cos_BTK, sin_BTK, wq, wk, wv, wo, q_norm_w, k_norm_w,
           segment_ids_BT=None, position_ids_BT=None, **_unused):
    from concourse.bass_utils import run_bass_kernel_spmd

    in_maps = make_in_maps(
        np.asarray(hidden_BTD, np.float32), np.asarray(cos_BTK, np.float32),
        np.asarray(sin_BTK, np.float32), np.asarray(wq, np.float32),
        np.asarray(wk, np.float32), np.asarray(wv, np.float32),
        np.asarray(wo, np.float32), np.asarray(q_norm_w, np.float32),
        np.asarray(k_norm_w, np.float32))
    nc = _get_nc()
    trace = bool(int(os.environ.get("BASS_KERNEL_TRACE", "0")))
    if trace:
        _install_ntff_hook()
    res = run_bass_kernel_spmd(nc, in_maps, core_ids=list(range(N_CORES)),
                               trace=trace)
    out = np.zeros((B, T, D), np.float32)
    for core in range(N_CORES):
        out[core // NKV] += res.results[core]["out_t"].astype(np.float32).T
        out[core // NKV, (CH - 1) * CT:] += \
            res.results[core]["out2"].astype(np.float32).T
    kernel.last_exec_time_ns = res.exec_time_ns
    kernel.last_results = res
    return out


kernel.last_exec_time_ns = None
kernel.last_results = None


# revision 35
# speedup vs baseline: 1.2558x; 1.0210x over previous
"""Trainium2 Bass kernel for gated GQA attention (nn_Attention_6476810683032).

Sharding: 8 cores = 2 (batch DP) x 4 (head-group TP).
Core c handles batch b=c//4, head group g=c%4 (q-heads 4g..4g+3, kv-head g).
Each core computes a partial o_proj output [D, T] (its 4 heads' contribution,
transposed layout); the host sums the 4 partials per batch and transposes.

On-device per core (all matmuls bf16 with fp32 PSUM accumulation):
  - projections from host-pre-transposed hidden_t [D, T] (channel-major
    outputs for q/gate/k, token-major for v) -- no on-device transposes
  - RMS norm via ones-matmul partition reduction + K=1 broadcast matmul
  - RoPE via partition-offset elementwise ops with a pre-signed sin table
  - causal attention in transposed-score form: S_T[tk,tq] = k_rot.T@q_rot,
    exp without max subtraction (logits bounded by the RMS norms)
  - softmax denominator: E tiles accumulate in SBUF on the DVE (bf16
    ping-pong adds stay in the 2x packed mode; partial-width diagonal
    steps add in place), then ONE ones-matmul per (chunk, head) -- saves
    ~60k PE columns (~30us) vs a per-m-step ones-matmul
  - sigmoid folded into the normalization: attn*sig(g)/den ==
    attn/((1+e^-g)*den); e^-g shares the exp ACT table, so the whole
    kernel uses Exp + Abs_reciprocal_sqrt only (9 table loads; eg exps
    are dep-ordered after the chunk's absr chain to prevent interleave)
  - partial o_proj: out_T[dout,t] = wo_slice.T @ gated (bf16 partials,
    summed in f32 on the host); the last chunk's heads-0/1 half goes to
    out2 as PE filler during the pair-(2,3) m-loop

Scheduling notes (PE p-state drops to 1.2GHz after any bubble and takes
~3us of continuous execution to return to 2.4GHz, so PE density is worth
more than engine-local optima; the chip also thermally throttles ~18%
run-to-run -- compare cold first-exec numbers or LDWEIGHTS-normalized):
  - hid loads in half-T tiles; pass A (k/v chunks 0-1 + head-0 q/gate)
    paces with the half-0 stripe; wqq/wqg are head-major 32KB tiles so
    each head's projection starts as its 0.5MB block lands
  - chunk 0 is fully reordered around the DMA stream: q-projections +
    all chains (wqq in-stripe), pair-(0,1) m-loop, pass B (k/v chunks
    2-3 from half 1), gate projections (wqg lands last), gating drains
  - input DMAs issue in first-use order, striped across the three rings
    (sync/scalar HWDGE + gpsimd SWDGE) balanced by bytes
  - o_proj of chunk c-1 + chunk-1 prefetch + last-chunk half o_proj are
    drip-fed between m-steps and into the drains as PE filler
  - final o_proj drains through 4 osb buffers, casts split DVE/ACT,
    writes striped across all three DMA rings
Measured: 419us baseline -> 376us (cold-clock best; ~81% PE occupancy).
"""

import os
import sys
from contextlib import ExitStack

import numpy as np

sys.path.insert(0, "/opt/trn_rl_repo")

import ml_dtypes  # noqa: E402

import concourse.bass as bass  # noqa: E402
import concourse.mybir as mybir  # noqa: E402
import concourse.tile as tile  # noqa: E402
from concourse import bacc  # noqa: E402
from concourse import masks as masks_mod  # noqa: E402

F32 = mybir.dt.float32
BF16 = mybir.dt.bfloat16
AF = mybir.ActivationFunctionType
ALU = mybir.AluOpType
BF = ml_dtypes.bfloat16

P = 128
B, T, D = 2, 2048, 2048
NH, NKV, HD = 16, 4, 128
NHL = NH // NKV          # local q heads per core (4)
CH = 4                   # tq chunks
CT = T // CH             # 512 tokens per chunk
TH = T // 2              # half-T (hid DMA tile width)
DT = D // P              # 16 contraction tiles
KT = T // P              # 16 tk tiles
EPS = 1e-6
SCALE = HD ** -0.5
N_CORES = 8


def _norm_rope(nc, pools, psr, pss, ones_col, ones_row, eps_t, x_bf, w_ap,
               cos_sl, sin_sl, out_ap, n):
    """RMS-norm (over partitions) + RoPE on a [128, n] channel-major tile.

    x_bf: [128, n] bf16 SBUF (pre-norm channels-on-partitions tile)
    w_ap: [128, 1] f32 norm weight
    cos_sl/sin_sl: [128, n] bf16 (sin pre-signed: rows 0-63 negated)
    out_ap: [128, n] bf16 destination
    """
    sbw, sbr = pools
    xsq = sbw.tile([P, n], BF16, tag="tmpa", name="xsq")
    nc.vector.tensor_tensor(xsq[:], x_bf, x_bf, op=ALU.mult)
    ssq = psr.tile([1, n], F32, tag="row", name="ssq")
    nc.tensor.matmul(ssq[:], ones_col, xsq[:], start=True, stop=True)
    rsq = sbr.tile([1, n], BF16, tag="rsq", name="rsq")
    absr = nc.scalar.activation(rsq[:], ssq[:], AF.Abs_reciprocal_sqrt,
                                scale=1.0 / HD, bias=eps_t)
    rb = pss.tile([P, n], F32, tag="ss", name="rb")
    nc.tensor.matmul(rb[:], ones_row, rsq[:], start=True, stop=True)
    wr = sbw.tile([P, n], BF16, tag="tmpb", name="wr")
    nc.vector.tensor_scalar(wr[:], rb[:], w_ap, None, ALU.mult)
    xn = sbw.tile([P, n], BF16, tag="xn", name="xn")
    nc.vector.tensor_tensor(xn[:], x_bf, wr[:], op=ALU.mult)
    t1 = sbw.tile([P, n], BF16, tag="tmpb", name="t1")
    nc.vector.tensor_tensor(t1[:], xn[:], cos_sl, op=ALU.mult)
    h = HD // 2
    xs = sbw.tile([P, n], BF16, tag="tmpc", name="xs", bufs=2)
    nc.vector.tensor_copy(xs[0:h, :], xn[h:P, :])
    nc.vector.tensor_copy(xs[h:P, :], xn[0:h, :])
    t2 = sbw.tile([P, n], BF16, tag="tmpa", name="t2")
    nc.vector.tensor_tensor(t2[:], xs[:], sin_sl, op=ALU.mult)
    nc.vector.tensor_tensor(out_ap, t1[:], t2[:], op=ALU.add)
    return absr


def build_nc():
    nc = bacc.Bacc("TRN2", target_bir_lowering=False, debug=False,
                   num_devices=N_CORES)
    hid_d = nc.dram_tensor("hid", [D, T], BF16, kind="ExternalInput")
    wqq_d = nc.dram_tensor("wqq", [NHL * D, HD], BF16, kind="ExternalInput")
    wqg_d = nc.dram_tensor("wqg", [NHL * D, HD], BF16, kind="ExternalInput")
    wk_d = nc.dram_tensor("wk", [D, HD], BF16, kind="ExternalInput")
    wv_d = nc.dram_tensor("wv", [D, HD], BF16, kind="ExternalInput")
    wo_d = nc.dram_tensor("wo", [NHL * HD, D], BF16, kind="ExternalInput")
    cos_d = nc.dram_tensor("cost", [P, T], BF16, kind="ExternalInput")
    sin_d = nc.dram_tensor("sinpm", [P, T], BF16, kind="ExternalInput")
    qw_d = nc.dram_tensor("qw", [P, 1], F32, kind="ExternalInput")
    kw_d = nc.dram_tensor("kw", [P, 1], F32, kind="ExternalInput")
    mask_d = nc.dram_tensor("masks", [P, P], BF16, kind="ExternalInput")
    out_d = nc.dram_tensor("out_t", [D, T], BF16, kind="ExternalOutput")
    # heads 0-1's half of the last chunk's o_proj (host adds the two halves)
    out2_d = nc.dram_tensor("out2", [D, CT], BF16, kind="ExternalOutput")

    with tile.TileContext(nc) as tc, ExitStack() as ctx, \
            nc.allow_low_precision(reason="bf16 softmax temps validated by rel_err"):
        sbp = ctx.enter_context(tc.tile_pool(name="sbp", bufs=1))
        sbw = ctx.enter_context(tc.tile_pool(name="sbw", bufs=3))
        sbr = ctx.enter_context(tc.tile_pool(name="sbr", bufs=2))
        sbq = ctx.enter_context(tc.tile_pool(name="sbq", bufs=6))
        psp = ctx.enter_context(tc.tile_pool(name="psp", bufs=2, space="PSUM"))
        pss = ctx.enter_context(tc.tile_pool(name="pss", bufs=2, space="PSUM"))
        psa = ctx.enter_context(tc.tile_pool(name="psa", bufs=2, space="PSUM"))
        psr = ctx.enter_context(tc.tile_pool(name="psr", bufs=2, space="PSUM"))

        # ---- persistent tiles ----
        wkT = [sbp.tile([P, HD], BF16, tag=f"wk{i}", name=f"wk{i}")
               for i in range(DT)]
        wvT = [sbp.tile([P, HD], BF16, tag=f"wv{i}", name=f"wv{i}")
               for i in range(DT)]
        hidh = [[sbp.tile([P, TH], BF16, tag=f"hid{d}_{j}", name=f"hid{d}_{j}")
                 for j in range(2)] for d in range(DT)]
        wqqt = [[sbp.tile([P, HD], BF16, tag=f"wqq{h}_{d}",
                         name=f"wqq{h}_{d}") for d in range(DT)]
                for h in range(NHL)]
        wqgt = [[sbp.tile([P, HD], BF16, tag=f"wqg{h}_{d}",
                         name=f"wqg{h}_{d}") for d in range(DT)]
                for h in range(NHL)]
        wo = [sbp.tile([P, D], BF16, tag=f"wo{i}", name=f"wo{i}")
              for i in range(NHL)]
        cost = sbp.tile([P, T], BF16, tag="cost")
        sinpm = sbp.tile([P, T], BF16, tag="sinpm")
        masks = sbp.tile([P, P], BF16, tag="masks")
        qw = sbp.tile([P, 1], F32, tag="qw")
        kw = sbp.tile([P, 1], F32, tag="kw")

        def hidsl(d, c):
            j, o = divmod(c, 2)
            return hidh[d][j][:, o * CT:(o + 1) * CT]

        # ---- DMA issue: first-use order, striped across the 3 rings ----
        rings = [nc.sync, nc.scalar, nc.gpsimd]
        ring_bytes = [0, 0, 0]

        def load(dst, src):
            i = ring_bytes.index(min(ring_bytes))
            rings[i].dma_start(dst, src)
            ring_bytes[i] += dst.size() * 2

        nc.gpsimd.dma_start(qw[:], qw_d[:, :])
        nc.gpsimd.dma_start(kw[:], kw_d[:, :])
        nc.gpsimd.dma_start(masks[:], mask_d[:, :])
        ring_bytes[2] += 64 * 1024
        for d in range(DT):
            ds_ = slice(d * P, (d + 1) * P)
            load(wkT[d][:], wk_d[ds_, :])
            load(wvT[d][:], wv_d[ds_, :])
            load(hidh[d][0][:], hid_d[ds_, 0:TH])
            # head-0 q/gate weights ride the stripe (pass A needs them)
            load(wqqt[0][d][:], wqq_d[d * P:(d + 1) * P, :])
            load(wqgt[0][d][:], wqg_d[d * P:(d + 1) * P, :])
            # rope-table chunk slices just ahead of their chains' first use
            if d == 3:
                load(cost[:, 0:CT], cos_d[:, 0:CT])
                load(sinpm[:, 0:CT], sin_d[:, 0:CT])
            if d == 11:
                load(cost[:, CT:TH], cos_d[:, CT:TH])
                load(sinpm[:, CT:TH], sin_d[:, CT:TH])
        for h in range(1, NHL):
            for d in range(DT):
                ds_ = slice((h * D + d * P), (h * D + (d + 1) * P))
                load(wqqt[h][d][:], wqq_d[ds_, :])
        for d in range(DT):
            ds_ = slice(d * P, (d + 1) * P)
            load(hidh[d][1][:], hid_d[ds_, TH:T])
        for h in range(1, NHL):
            for d in range(DT):
                ds_ = slice((h * D + d * P), (h * D + (d + 1) * P))
                load(wqgt[h][d][:], wqg_d[ds_, :])
        load(cost[:, TH:T], cos_d[:, TH:T])
        load(sinpm[:, TH:T], sin_d[:, TH:T])
        for i in range(NHL):
            load(wo[i][:], wo_d[i * P:(i + 1) * P, :])

        ones_col = sbp.tile([P, 1], BF16, tag="ones_col")
        nc.vector.memset(ones_col[:], 1.0)
        ones_row = sbp.tile([1, P], BF16, tag="ones_row")
        nc.vector.memset(ones_row[:], 1.0)
        eps_t = sbp.tile([1, 1], F32, tag="eps_t")
        nc.vector.memset(eps_t[:], EPS)
        ident = sbp.tile([P, P], BF16, tag="ident")
        masks_mod.make_identity(nc, ident[:])
        krot = sbp.tile([P, T], BF16, tag="krot")
        vct = sbp.tile([P, T], BF16, tag="vct")
        vsb = []
        for i in range(KT):
            vsb.append(sbp.tile([P, HD], BF16, tag=f"v{i}", name=f"v{i}"))

        kbfs = []

        # ---- phase 0 pass A: k/v chunks 0-1 + chunk-0 q/gate, d-outer so
        # the PE paces with the hid half-0 DMA stream.
        def proj_pass(half, chunks, qg_psum):
            kps = [pss.tile([P, CT], F32, tag="ss", name=f"kps{c}")
                   for c in chunks]
            vps = [psr.tile([P, CT], F32, tag="row", name=f"vps{c}")
                   for c in chunks]
            for d in range(DT):
                st, sp = (d == 0), (d == DT - 1)
                for i, c in enumerate(chunks):
                    nc.tensor.matmul(kps[i][:], wkT[d][:, :], hidsl(d, c),
                                     start=st, stop=sp)
                    nc.tensor.matmul(vps[i][:], wvT[d][:, :], hidsl(d, c),
                                     start=st, stop=sp)
                if qg_psum is not None:
                    nc.tensor.matmul(qg_psum[0][:], wqqt[0][d][:],
                                     hidsl(d, 0), start=st, stop=sp)
                    nc.tensor.matmul(qg_psum[1][:], wqgt[0][d][:],
                                     hidsl(d, 0), start=st, stop=sp)
            for i, c in enumerate(chunks):
                kbf = sbw.tile([P, CT], BF16, tag="kbf", name="kbf", bufs=4)
                nc.vector.tensor_copy(kbf[:], kps[i][:])
                kbfs.append(kbf)
                nc.vector.tensor_copy(vct[:, c * CT:(c + 1) * CT], vps[i][:])
            for tt in range(chunks[0] * 4, (chunks[-1] + 1) * 4):
                tps = pss.tile([P, P], BF16, tag="ss", name="tps")
                nc.tensor.transpose(tps[:], vct[:, tt * P:(tt + 1) * P],
                                    ident[:])
                nc.vector.tensor_copy(vsb[tt][:], tps[:])

        qp0 = psp.tile([P, CT], F32, tag="pp", name="qp0")
        gp0 = psp.tile([P, CT], F32, tag="pp", name="gp0")
        proj_pass(0, (0, 1), (qp0, gp0))
        q_sb0 = sbq.tile([P, CT], BF16, tag="q_sb", bufs=4, name="q_sb0")
        nc.vector.tensor_copy(q_sb0[:], qp0[:])
        g_sb0 = sbq.tile([P, CT], BF16, tag="g_sb", bufs=5, name="g_sb0")
        nc.vector.tensor_copy(g_sb0[:], gp0[:])
        pre_pairs = {0: (q_sb0, g_sb0)}

        # ---- phase 1: per tq-chunk: q/gate proj, attention ----
        # o_proj for chunk c-1 is emitted after chunk c's norm chains so the
        # PE has dense work while the chains' DVE/ACT latency drains.
        def _o_proj(oc, og, heads=tuple(range(NHL))):
            ocs = slice(oc * CT, (oc + 1) * CT)
            # the final o_proj has the PSUM to itself: rotate all four tag
            # groups so psum-free never gates the accumulation chains
            pools4 = [(psp, "pp"), (pss, "ss"), (psa, "aa"), (psr, "row")]
            for dt in range(DT):
                ds_ = slice(dt * P, (dt + 1) * P)
                pl, tg = pools4[dt % 4]
                pso = pl.tile([P, CT], F32, tag=tg, name="pso")
                for i2, ct4 in enumerate(heads):
                    nc.tensor.matmul(pso[:], wo[ct4][:, ds_], og[ct4][:],
                                     start=(i2 == 0),
                                     stop=(i2 == len(heads) - 1))
                osb = sbw.tile([P, CT], BF16, tag="osb", bufs=4, name="osb")
                if dt % 2 == 0:
                    nc.vector.tensor_copy(osb[:], pso[:])
                else:
                    nc.scalar.copy(osb[:], pso[:])
                rings[dt % 3].dma_start(out_d[ds_, ocs], osb[:])

        prev_gated = None
        for c in range(CH):
            cs = slice(c * CT, (c + 1) * CT)
            q_sbs = []
            g_sbs = []
            sigs = []
            qrots = []

            chain_absr = []

            def _q_chain(h, c=None, cs=None, q_sbs=None, qrots=None):
                qrot = sbw.tile([P, CT], BF16, tag="qrot", bufs=4,
                                name="qrot")
                a = _norm_rope(nc, (sbw, sbr), psr, pss, ones_col[:],
                               ones_row[:], eps_t[:], q_sbs[h][:], qw[:],
                               cost[:, cs], sinpm[:, cs], qrot[:], CT)
                qrots.append(qrot)
                chain_absr.append(a)

            def _proj(w_t, h, dst_tag, bufs_):
                ps = psp.tile([P, CT], F32, tag="pp")
                for d in range(DT):
                    nc.tensor.matmul(ps[:], w_t[h][d][:], hidsl(d, c),
                                     start=(d == 0), stop=(d == DT - 1))
                sb = sbq.tile([P, CT], BF16, tag=dst_tag, bufs=bufs_)
                nc.vector.tensor_copy(sb[:], ps[:])
                return sb

            for h in range(NHL):
                if c == 0:
                    break
                if h == 0 and c in pre_pairs:
                    q_sbs.append(pre_pairs[c][0])
                    g_sbs.append(pre_pairs[c][1])
                    chain_absr.append(_norm_rope(
                        nc, (sbw, sbr), psr, pss, ones_col[:],
                        ones_row[:], eps_t[:], kbfs[c][:], kw[:],
                        cost[:, cs], sinpm[:, cs], krot[:, cs], CT))
                    continue
                ps = psp.tile([P, CT], F32, tag="pp")
                for d in range(DT):
                    nc.tensor.matmul(ps[:], wqqt[h][d][:], hidsl(d, c),
                                     start=(d == 0), stop=(d == DT - 1))
                q_sb = sbq.tile([P, CT], BF16, tag="q_sb", bufs=4)
                nc.vector.tensor_copy(q_sb[:], ps[:])
                q_sbs.append(q_sb)
                ps2 = psp.tile([P, CT], F32, tag="pp")
                for d in range(DT):
                    nc.tensor.matmul(ps2[:], wqgt[h][d][:], hidsl(d, c),
                                     start=(d == 0), stop=(d == DT - 1))
                g_sb = sbq.tile([P, CT], BF16, tag="g_sb", bufs=5)
                nc.vector.tensor_copy(g_sb[:], ps2[:])
                g_sbs.append(g_sb)
                # sandwich a norm/rope chain after each proj pair so the
                # chain's DVE/ACT latency hides behind the next pair's mms
                if h == 0:
                    chain_absr.append(_norm_rope(
                        nc, (sbw, sbr), psr, pss, ones_col[:],
                        ones_row[:], eps_t[:], kbfs[c][:], kw[:],
                        cost[:, cs], sinpm[:, cs], krot[:, cs], CT))
                else:
                    _q_chain(h - 1, c=c, cs=cs, q_sbs=q_sbs, qrots=qrots)
            if c != 0:
                _q_chain(NHL - 1, c=c, cs=cs, q_sbs=q_sbs, qrots=qrots)

            def _emit_sigs():
                for h in range(NHL):
                    # e^-g; the sigmoid is folded into the softmax
                    # denominator: attn*sig(g)/den == attn/((1+e^-g)*den)
                    sig = sbq.tile([P, CT], BF16, tag="sig", bufs=4,
                                   name="sig")
                    si = nc.scalar.activation(sig[:], g_sbs[h][:], AF.Exp,
                                              scale=-1.0)
                    # keep the chunk's absr chain ops contiguous on ACT:
                    # each exp<->absr interleave costs a ~1.3us table load
                    bass._add_dep_helper(si.ins, chain_absr[-1].ins,
                                         sync=False,
                                         reason="group exps after absr")
                    sigs.append(sig)
            gated = []
            nm = 4 * c + 4
            # Filler work drip-fed between attention m-steps keeps the PE
            # dense while ACT runs the exps: o_proj(c-1) tiles, and for
            # chunk 0 (which has no prior o_proj) the next chunk's first
            # projection pair.
            fillers = []
            if prev_gated is not None:
                ocs = slice((c - 1) * CT, c * CT)

                def _mk_oproj(dt, ocs=ocs, og=prev_gated):
                    def run():
                        ds_ = slice(dt * P, (dt + 1) * P)
                        pso = psp.tile([P, CT], F32, tag="pp", name="pso")
                        for ct4 in range(NHL):
                            nc.tensor.matmul(pso[:], wo[ct4][:, ds_],
                                             og[ct4][:], start=(ct4 == 0),
                                             stop=(ct4 == NHL - 1))
                        osb = sbw.tile([P, CT], BF16, tag="osb", bufs=4,
                                       name="osb")
                        nc.vector.tensor_copy(osb[:], pso[:])
                        nc.sync.dma_start(out_d[ds_, ocs], osb[:])
                    return run
                fillers += [_mk_oproj(dt) for dt in range(DT)]
            fill = {"i": 0}
            n_steps = 2 * nm + 6

            def _fill_tick(step):
                due = min(len(fillers),
                          len(fillers) * (step + 1) // n_steps + 1)
                while fill["i"] < due:
                    fillers[fill["i"]]()
                    fill["i"] += 1

            step_no = [0]
            pair_state = {}

            def _attn_mloop(hp):
                pair = (hp, hp + 1)
                attns = {h: psa.tile([P, CT], F32, tag="aa",
                                     name=f"attn{h}") for h in pair}
                esums = {}
                prevE = None
                for m in range(nm):
                    ks = slice(m * P, (m + 1) * P)
                    r = m - 4 * c
                    lo = P * r if r > 0 else 0
                    ns = slice(lo, CT)
                    Es = {}
                    for h in pair:
                        sps = pss.tile([P, CT], F32, tag="ss", name="sps")
                        nc.tensor.matmul(sps[:, ns], krot[:, ks],
                                         qrots[h][:, ns],
                                         start=True, stop=True)
                        E = sbw.tile([P, CT], BF16, tag="E", name="E",
                                     bufs=6)
                        nc.scalar.activation(E[:, ns], sps[:, ns], AF.Exp,
                                             scale=SCALE)
                        if r >= 0:
                            nc.gpsimd.tensor_tensor(
                                E[:, lo:lo + P], E[:, lo:lo + P],
                                masks[:, 0:P], op=ALU.mult)
                        Es[h] = E
                    _fill_tick(step_no[0])
                    step_no[0] += 1
                    for h in pair:
                        # running softmax denominator in SBUF: out-of-place
                        # ping-pong adds keep the DVE in 2x packed mode; the
                        # diagonal (partial-width) steps accumulate in place
                        # to preserve the untouched columns
                        if m == 0 and c == 0:
                            # chunk 0's later steps accumulate in place, so
                            # the running sum must not pin an E-ring slot
                            ne = sbw.tile([P, CT], BF16, tag="esum",
                                          bufs=4, name="esum")
                            nc.vector.tensor_copy(ne[:], Es[h][:])
                            esums[h] = ne
                        elif m == 0:
                            esums[h] = Es[h]
                        elif r > 0:
                            nc.vector.tensor_tensor(
                                esums[h][:, ns], esums[h][:, ns],
                                Es[h][:, ns], op=ALU.add)
                        else:
                            ne = sbw.tile([P, CT], BF16, tag="esum",
                                          bufs=4, name="esum")
                            nc.vector.tensor_tensor(ne[:], esums[h][:],
                                                    Es[h][:], op=ALU.add)
                            esums[h] = ne
                    # PV matmuls run one m-step behind the scores/exp so
                    # each exp has a full step of PE work to hide behind
                    if prevE is not None:
                        pm, pns, pEs = prevE
                        for h in pair:
                            nc.tensor.matmul(attns[h][:, pns], vsb[pm][:],
                                             pEs[h][:, pns],
                                             start=(pm == 0), stop=False)
                    prevE = (m, ns, Es)
                pm, pns, pEs = prevE
                for h in pair:
                    nc.tensor.matmul(attns[h][:, pns], vsb[pm][:],
                                     pEs[h][:, pns], start=(pm == 0),
                                     stop=True)
                _fill_tick(step_no[0])
                step_no[0] += 1
                pair_state[hp] = (attns, esums)

            def _attn_drain(hp):
                pair = (hp, hp + 1)
                attns, esums = pair_state.pop(hp)
                dpss = {}
                for h in pair:
                    dps = psr.tile([1, CT], F32, tag="row", name="dps")
                    nc.tensor.matmul(dps[:], ones_col[:], esums[h][:],
                                     start=True, stop=True)
                    dpss[h] = dps
                _fill_tick(step_no[0])
                step_no[0] += 1
                for h in pair:
                    dn = sbr.tile([1, CT], BF16, tag="rowtmp", name="dn")
                    nc.vector.tensor_copy(dn[:], dpss[h][:])
                    rb = psr.tile([P, CT], F32, tag="row", name="rb")
                    nc.tensor.matmul(rb[:], ones_row[:], dn[:],
                                     start=True, stop=True)
                    gd = sbw.tile([P, CT], F32, tag="rcb", name="gd",
                                  bufs=2)
                    nc.vector.scalar_tensor_tensor(gd[:], sigs[h][:], 1.0,
                                                   rb[:], op0=ALU.add,
                                                   op1=ALU.mult)
                    rcb = sbw.tile([P, CT], F32, tag="rcb", name="rcb",
                                   bufs=2)
                    nc.vector.reciprocal_approx_fast(out=rcb[:], in_=gd[:])
                    g = sbq.tile([P, CT], BF16, tag="gated", bufs=8)
                    nc.vector.tensor_tensor(g[:], attns[h][:], rcb[:],
                                            op=ALU.mult)
                    gated.append(g)
                _fill_tick(step_no[0])
                step_no[0] += 1
                if c == CH - 1 and hp == 0:
                    # heads 0-1's half of this chunk's o_proj can run as
                    # fillers during pair (2,3); host adds out2 to out_t
                    g01 = list(gated)

                    def _mk_ohalf(dt, g01=g01):
                        def run():
                            ds_ = slice(dt * P, (dt + 1) * P)
                            pso = psp.tile([P, CT], F32, tag="pp",
                                           name="psoh")
                            for i2 in range(2):
                                nc.tensor.matmul(pso[:], wo[i2][:, ds_],
                                                 g01[i2][:], start=(i2 == 0),
                                                 stop=(i2 == 1))
                            osb = sbw.tile([P, CT], BF16, tag="osb", bufs=4,
                                           name="osbh")
                            nc.vector.tensor_copy(osb[:], pso[:])
                            nc.sync.dma_start(out2_d[ds_, :], osb[:])
                        return run
                    fillers.extend(_mk_ohalf(dt) for dt in range(DT))

            if c == 0:
                # wqg trails the DMA stripe: all q-projections + chains
                # first (wqq rides the stripe), then pair (0,1) attention
                # runs while the wqg stream lands, then gate projections
                q_sbs.append(pre_pairs[0][0])
                g_sbs.append(pre_pairs[0][1])
                q_sbs.append(_proj(wqqt, 1, "q_sb", 4))
                chain_absr.append(_norm_rope(
                    nc, (sbw, sbr), psr, pss, ones_col[:], ones_row[:],
                    eps_t[:], kbfs[0][:], kw[:], cost[:, cs], sinpm[:, cs],
                    krot[:, cs], CT))
                q_sbs.append(_proj(wqqt, 2, "q_sb", 4))
                _q_chain(0, c=c, cs=cs, q_sbs=q_sbs, qrots=qrots)
                q_sbs.append(_proj(wqqt, 3, "q_sb", 4))
                _q_chain(1, c=c, cs=cs, q_sbs=q_sbs, qrots=qrots)
                _q_chain(2, c=c, cs=cs, q_sbs=q_sbs, qrots=qrots)
                _q_chain(3, c=c, cs=cs, q_sbs=q_sbs, qrots=qrots)
                _attn_mloop(0)
                # phase 0 pass B: k/v for chunks 2-3 from hid half 1 --
                # wqg-independent PE work that covers the gate-weight DMA
                proj_pass(1, (2, 3), None)
                g_sbs.append(_proj(wqgt, 1, "g_sb", 5))
                g_sbs.append(_proj(wqgt, 2, "g_sb", 5))
                g_sbs.append(_proj(wqgt, 3, "g_sb", 5))
                # chunk-1 first-pair prefetch, drip-fed as pair-(2,3) filler
                qp1 = psp.tile([P, CT], F32, tag="pp", name="qp1")
                gp1 = psp.tile([P, CT], F32, tag="pp", name="gp1")

                def _mk_proj(ps_t, w_t, dlist):
                    def run():
                        for d in dlist:
                            nc.tensor.matmul(
                                ps_t[:], w_t[0][d][:], hidsl(d, 1),
                                start=(d == 0), stop=(d == DT - 1))
                    return run
                for d0 in range(0, DT, 4):
                    fillers.append(_mk_proj(qp1, wqqt, range(d0, d0 + 4)))
                for d0 in range(0, DT, 4):
                    fillers.append(_mk_proj(gp1, wqgt, range(d0, d0 + 4)))
                _emit_sigs()
                _attn_drain(0)
                _attn_mloop(2)
                _attn_drain(2)
            else:
                _emit_sigs()
                for hp in (0, 2):
                    _attn_mloop(hp)
                    _attn_drain(hp)

            while fill["i"] < len(fillers):
                fillers[fill["i"]]()
                fill["i"] += 1
            if c == 0:
                q_sb1 = sbq.tile([P, CT], BF16, tag="q_sb", bufs=4,
                                 name="q_sb1")
                nc.vector.tensor_copy(q_sb1[:], qp1[:])
                g_sb1 = sbq.tile([P, CT], BF16, tag="g_sb", bufs=5,
                                 name="g_sb1")
                nc.vector.tensor_copy(g_sb1[:], gp1[:])
                pre_pairs[1] = (q_sb1, g_sb1)
            prev_gated = gated
        _o_proj(CH - 1, prev_gated, heads=(2, 3))
    nc.compile()
    return nc


def make_in_maps(hidden, cos, sin, wq, wk, wv, wo, q_norm_w, k_norm_w):
    """Build the 8 per-core input maps (host-side sharding + layout prep)."""
    i_idx = np.arange(P)[:, None]
    j_idx = np.arange(P)[None, :]
    masks = (j_idx >= i_idx).astype(BF)
    in_maps = []
    for core in range(N_CORES):
        b, g = core // NKV, core % NKV
        heads = range(NHL * g, NHL * g + NHL)
        sin_t = sin[b].T.copy()
        sin_t[:HD // 2] = -sin_t[:HD // 2]
        in_maps.append({
            "hid": np.ascontiguousarray(hidden[b].T).astype(BF),
            "wqq": np.concatenate(
                [wq[:, h * 2 * HD: h * 2 * HD + HD] for h in heads], 0
            ).astype(BF),
            "wqg": np.concatenate(
                [wq[:, h * 2 * HD + HD: (h + 1) * 2 * HD] for h in heads], 0
            ).astype(BF),
            "wk": np.ascontiguousarray(wk[:, g * HD:(g + 1) * HD]).astype(BF),
            "wv": np.ascontiguousarray(wv[:, g * HD:(g + 1) * HD]).astype(BF),
            "wo": np.ascontiguousarray(
                wo[NHL * HD * g: NHL * HD * (g + 1), :]).astype(BF),
            "cost": np.ascontiguousarray(cos[b].T).astype(BF),
            "sinpm": np.ascontiguousarray(sin_t).astype(BF),
            "qw": np.ascontiguousarray(q_norm_w[:, None]).astype(np.float32),
            "kw": np.ascontiguousarray(k_norm_w[:, None]).astype(np.float32),
            "masks": np.ascontiguousarray(masks),
        })
    return in_maps


def _install_ntff_hook():
    """Inject antenv.axon_hooks with a ctypes NTFF profile hook.

    The container's antenv package lacks axon_hooks, so bass_utils'
    trace=True path can't find the hook. Replicates the boot script's
    _ntff_profile_via_ctypes against libaxon_pjrt.so.
    """
    import contextlib
    import ctypes
    import types

    if "antenv.axon_hooks" in sys.modules:
        return
    lib = None
    for so_path in ("/opt/axon/libaxon_pjrt.so",
                    "/root/.axon_site/axon/libaxon_pjrt.so"):
        try:
            lib = ctypes.CDLL(so_path)
            break
        except OSError:
            continue
    if lib is None:
        return
    if not hasattr(lib, "axon_start_nrt_profile"):
        return
    lib.axon_start_nrt_profile.argtypes = [ctypes.POINTER(ctypes.c_int64),
                                           ctypes.c_size_t]
    lib.axon_start_nrt_profile.restype = ctypes.c_int64
    lib.axon_stop_nrt_profile.argtypes = [ctypes.c_char_p]
    lib.axon_stop_nrt_profile.restype = ctypes.c_int64

    @contextlib.contextmanager
    def _hook(output_dir, device_ids):
        import jax

        jax.devices()
        if device_ids:
            ids = (ctypes.c_int64 * len(device_ids))(*device_ids)
            rc = lib.axon_start_nrt_profile(ids, len(device_ids))
        else:
            rc = lib.axon_start_nrt_profile(None, 0)
        if rc != 0:
            raise RuntimeError(f"axon_start_nrt_profile rc={rc}")
        try:
            yield
        finally:
            n = lib.axon_stop_nrt_profile(str(output_dir).encode())
            print(f"profile: {n} file(s) written to {output_dir}",
                  file=sys.stderr)

    m = types.ModuleType("antenv.axon_hooks")
    m.get_axon_ntff_profile_hook = lambda: _hook
    m.set_axon_ntff_profile_hook = lambda h: None
    sys.modules["antenv.axon_hooks"] = m


_NC_CACHE = None


def _get_nc():
    global _NC_CACHE
    if _NC_CACHE is None:
        _NC_CACHE = build_nc()
    return _NC_CACHE


def kernel(hidden_BTD, cos_BTK, sin_BTK, wq, wk, wv, wo, q_norm_w, k_norm_w,
           segment_ids_BT=None, position_ids_BT=None, **_unused):
    from concourse.bass_utils import run_bass_kernel_spmd

    in_maps = make_in_maps(
        np.asarray(hidden_BTD, np.float32), np.asarray(cos_BTK, np.float32),
        np.asarray(sin_BTK, np.float32), np.asarray(wq, np.float32),
        np.asarray(wk, np.float32), np.asarray(wv, np.float32),
        np.asarray(wo, np.float32), np.asarray(q_norm_w, np.float32),
        np.asarray(k_norm_w, np.float32))
    nc = _get_nc()
    trace = bool(int(os.environ.get("BASS_KERNEL_TRACE", "0")))
    if trace:
        _install_ntff_hook()
    res = run_bass_kernel_spmd(nc, in_maps, core_ids=list(range(N_CORES)),
                               trace=trace)
    out = np.zeros((B, T, D), np.float32)
    for core in range(N_CORES):
        out[core // NKV] += res.results[core]["out_t"].astype(np.float32).T
        out[core // NKV, (CH - 1) * CT:] += \
            res.results[core]["out2"].astype(np.float32).T
    kernel.last_exec_time_ns = res.exec_time_ns
    kernel.last_results = res
    return out


kernel.last_exec_time_ns = None
kernel.last_results = None


# revision 36
# speedup vs baseline: 1.2782x; 1.0178x over previous
"""Trainium2 Bass kernel for gated GQA attention (nn_Attention_6476810683032).

Sharding: 8 cores = 2 (batch DP) x 4 (head-group TP).
Core c handles batch b=c//4, head group g=c%4 (q-heads 4g..4g+3, kv-head g).
Each core computes a partial o_proj output [D, T] (its 4 heads' contribution,
transposed layout); the host sums the 4 partials per batch and transposes.

On-device per core (all matmuls bf16 with fp32 PSUM accumulation):
  - projections from host-pre-transposed hidden_t [D, T] (channel-major
    outputs for q/gate/k, token-major for v) -- no on-device transposes
  - RMS norm via ones-matmul partition reduction + K=1 broadcast matmul
  - RoPE via partition-offset elementwise ops with a pre-signed sin table
  - causal attention in transposed-score form: S_T[tk,tq] = k_rot.T@q_rot,
    exp without max subtraction (logits bounded by the RMS norms)
  - softmax denominator: E tiles accumulate in SBUF on the DVE (bf16
    ping-pong adds stay in the 2x packed mode; partial-width diagonal
    steps add in place), then ONE ones-matmul per (chunk, head) -- saves
    ~60k PE columns (~30us) vs a per-m-step ones-matmul
  - sigmoid folded into the normalization: attn*sig(g)/den ==
    attn/((1+e^-g)*den); e^-g shares the exp ACT table, so the whole
    kernel uses Exp + Abs_reciprocal_sqrt only (9 table loads; eg exps
    are dep-ordered after the chunk's absr chain to prevent interleave)
  - partial o_proj: out_T[dout,t] = wo_slice.T @ gated (bf16 partials,
    summed in f32 on the host); the last chunk's heads-0/1 half goes to
    out2 as PE filler during the pair-(2,3) m-loop

Scheduling notes (PE p-state drops to 1.2GHz after any bubble and takes
~3us of continuous execution to return to 2.4GHz, so PE density is worth
more than engine-local optima; the chip also thermally throttles ~18%
run-to-run -- compare cold first-exec numbers or LDWEIGHTS-normalized):
  - hid loads in half-T tiles; pass A (k/v chunks 0-1 + head-0 q/gate)
    paces with the half-0 stripe; wqq/wqg are head-major 32KB tiles so
    each head's projection starts as its 0.5MB block lands
  - chunk 0 is fully reordered around the DMA stream: q-projections +
    all chains (wqq in-stripe), pair-(0,1) m-loop, pass B (k/v chunks
    2-3 from half 1), gate projections (wqg lands last), gating drains
  - input DMAs issue in first-use order, striped across the three rings
    (sync/scalar HWDGE + gpsimd SWDGE) balanced by bytes
  - o_proj of chunk c-1 + chunk-1 prefetch + last-chunk half o_proj are
    drip-fed between m-steps and into the drains as PE filler
  - final o_proj drains through 4 osb buffers, casts split DVE/ACT,
    writes striped across all three DMA rings
Measured: 419us baseline -> 376us (cold-clock best; ~81% PE occupancy).
"""

import os
import sys
from contextlib import ExitStack

import numpy as np

sys.path.insert(0, "/opt/trn_rl_repo")

import ml_dtypes  # noqa: E402

import concourse.bass as bass  # noqa: E402
import concourse.mybir as mybir  # noqa: E402
import concourse.tile as tile  # noqa: E402
from concourse import bacc  # noqa: E402
from concourse import masks as masks_mod  # noqa: E402

F32 = mybir.dt.float32
BF16 = mybir.dt.bfloat16
AF = mybir.ActivationFunctionType
ALU = mybir.AluOpType
BF = ml_dtypes.bfloat16

P = 128
B, T, D = 2, 2048, 2048
NH, NKV, HD = 16, 4, 128
NHL = NH // NKV          # local q heads per core (4)
CH = 4                   # tq chunks
CT = T // CH             # 512 tokens per chunk
TH = T // 2              # half-T (hid DMA tile width)
DT = D // P              # 16 contraction tiles
KT = T // P              # 16 tk tiles
EPS = 1e-6
SCALE = HD ** -0.5
N_CORES = 8


def _norm_rope(nc, pools, psr, pss, ones_col, ones_row, eps_t, x_bf, w_ap,
               cos_sl, sin_sl, out_ap, n):
    """RMS-norm (over partitions) + RoPE on a [128, n] channel-major tile.

    x_bf: [128, n] bf16 SBUF (pre-norm channels-on-partitions tile)
    w_ap: [128, 1] f32 norm weight
    cos_sl/sin_sl: [128, n] bf16 (sin pre-signed: rows 0-63 negated)
    out_ap: [128, n] bf16 destination
    """
    sbw, sbr = pools
    xsq = sbw.tile([P, n], BF16, tag="tmpa", name="xsq")
    nc.vector.tensor_tensor(xsq[:], x_bf, x_bf, op=ALU.mult)
    ssq = psr.tile([1, n], F32, tag="row", name="ssq")
    nc.tensor.matmul(ssq[:], ones_col, xsq[:], start=True, stop=True)
    rsq = sbr.tile([1, n], BF16, tag="rsq", name="rsq")
    absr = nc.scalar.activation(rsq[:], ssq[:], AF.Abs_reciprocal_sqrt,
                                scale=1.0 / HD, bias=eps_t)
    rb = pss.tile([P, n], F32, tag="ss", name="rb")
    nc.tensor.matmul(rb[:], ones_row, rsq[:], start=True, stop=True)
    wr = sbw.tile([P, n], BF16, tag="tmpb", name="wr")
    nc.vector.tensor_scalar(wr[:], rb[:], w_ap, None, ALU.mult)
    xn = sbw.tile([P, n], BF16, tag="xn", name="xn")
    nc.vector.tensor_tensor(xn[:], x_bf, wr[:], op=ALU.mult)
    t1 = sbw.tile([P, n], BF16, tag="tmpb", name="t1")
    nc.vector.tensor_tensor(t1[:], xn[:], cos_sl, op=ALU.mult)
    h = HD // 2
    xs = sbw.tile([P, n], BF16, tag="tmpc", name="xs", bufs=2)
    nc.vector.tensor_copy(xs[0:h, :], xn[h:P, :])
    nc.vector.tensor_copy(xs[h:P, :], xn[0:h, :])
    t2 = sbw.tile([P, n], BF16, tag="tmpa", name="t2")
    nc.vector.tensor_tensor(t2[:], xs[:], sin_sl, op=ALU.mult)
    nc.vector.tensor_tensor(out_ap, t1[:], t2[:], op=ALU.add)
    return absr


def build_nc():
    nc = bacc.Bacc("TRN2", target_bir_lowering=False, debug=False,
                   num_devices=N_CORES)
    hid_d = nc.dram_tensor("hid", [D, T], BF16, kind="ExternalInput")
    wqq_d = nc.dram_tensor("wqq", [NHL * D, HD], BF16, kind="ExternalInput")
    wqg_d = nc.dram_tensor("wqg", [NHL * D, HD], BF16, kind="ExternalInput")
    wk_d = nc.dram_tensor("wk", [D, HD], BF16, kind="ExternalInput")
    wv_d = nc.dram_tensor("wv", [D, HD], BF16, kind="ExternalInput")
    wo_d = nc.dram_tensor("wo", [NHL * HD, D], BF16, kind="ExternalInput")
    cos_d = nc.dram_tensor("cost", [P, T], BF16, kind="ExternalInput")
    sin_d = nc.dram_tensor("sinpm", [P, T], BF16, kind="ExternalInput")
    qw_d = nc.dram_tensor("qw", [P, 1], F32, kind="ExternalInput")
    kw_d = nc.dram_tensor("kw", [P, 1], F32, kind="ExternalInput")
    mask_d = nc.dram_tensor("masks", [P, P], BF16, kind="ExternalInput")
    out_d = nc.dram_tensor("out_t", [D, T], BF16, kind="ExternalOutput")
    # heads 0-1's half of the last chunk's o_proj (host adds the two halves)
    out2_d = nc.dram_tensor("out2", [D, CT], BF16, kind="ExternalOutput")

    with tile.TileContext(nc) as tc, ExitStack() as ctx, \
            nc.allow_low_precision(reason="bf16 softmax temps validated by rel_err"):
        sbp = ctx.enter_context(tc.tile_pool(name="sbp", bufs=1))
        sbw = ctx.enter_context(tc.tile_pool(name="sbw", bufs=3))
        sbr = ctx.enter_context(tc.tile_pool(name="sbr", bufs=2))
        sbq = ctx.enter_context(tc.tile_pool(name="sbq", bufs=6))
        psp = ctx.enter_context(tc.tile_pool(name="psp", bufs=2, space="PSUM"))
        pss = ctx.enter_context(tc.tile_pool(name="pss", bufs=2, space="PSUM"))
        psa = ctx.enter_context(tc.tile_pool(name="psa", bufs=2, space="PSUM"))
        psr = ctx.enter_context(tc.tile_pool(name="psr", bufs=2, space="PSUM"))

        # ---- persistent tiles ----
        wkT = [sbp.tile([P, HD], BF16, tag=f"wk{i}", name=f"wk{i}")
               for i in range(DT)]
        wvT = [sbp.tile([P, HD], BF16, tag=f"wv{i}", name=f"wv{i}")
               for i in range(DT)]
        hidh = [[sbp.tile([P, TH], BF16, tag=f"hid{d}_{j}", name=f"hid{d}_{j}")
                 for j in range(2)] for d in range(DT)]
        wqqt = [[sbp.tile([P, HD], BF16, tag=f"wqq{h}_{d}",
                         name=f"wqq{h}_{d}") for d in range(DT)]
                for h in range(NHL)]
        wqgt = [[sbp.tile([P, HD], BF16, tag=f"wqg{h}_{d}",
                         name=f"wqg{h}_{d}") for d in range(DT)]
                for h in range(NHL)]
        wo = [sbp.tile([P, D], BF16, tag=f"wo{i}", name=f"wo{i}")
              for i in range(NHL)]
        cost = sbp.tile([P, T], BF16, tag="cost")
        sinpm = sbp.tile([P, T], BF16, tag="sinpm")
        masks = sbp.tile([P, P], BF16, tag="masks")
        qw = sbp.tile([P, 1], F32, tag="qw")
        kw = sbp.tile([P, 1], F32, tag="kw")

        def hidsl(d, c):
            j, o = divmod(c, 2)
            return hidh[d][j][:, o * CT:(o + 1) * CT]

        # ---- DMA issue: first-use order, striped across the 3 rings ----
        rings = [nc.sync, nc.scalar, nc.gpsimd]
        ring_bytes = [0, 0, 0]

        def load(dst, src):
            i = ring_bytes.index(min(ring_bytes))
            rings[i].dma_start(dst, src)
            ring_bytes[i] += dst.size() * 2

        nc.gpsimd.dma_start(qw[:], qw_d[:, :])
        nc.gpsimd.dma_start(kw[:], kw_d[:, :])
        nc.gpsimd.dma_start(masks[:], mask_d[:, :])
        ring_bytes[2] += 64 * 1024
        for d in range(DT):
            ds_ = slice(d * P, (d + 1) * P)
            load(wkT[d][:], wk_d[ds_, :])
            load(wvT[d][:], wv_d[ds_, :])
            load(hidh[d][0][:], hid_d[ds_, 0:TH])
            # head-0 q/gate weights ride the stripe (pass A needs them)
            load(wqqt[0][d][:], wqq_d[d * P:(d + 1) * P, :])
            load(wqgt[0][d][:], wqg_d[d * P:(d + 1) * P, :])
            # rope-table chunk slices just ahead of their chains' first use
            if d == 3:
                load(cost[:, 0:CT], cos_d[:, 0:CT])
                load(sinpm[:, 0:CT], sin_d[:, 0:CT])
            if d == 11:
                load(cost[:, CT:TH], cos_d[:, CT:TH])
                load(sinpm[:, CT:TH], sin_d[:, CT:TH])
        for h in range(1, NHL):
            for d in range(DT):
                ds_ = slice((h * D + d * P), (h * D + (d + 1) * P))
                load(wqqt[h][d][:], wqq_d[ds_, :])
        # gate weights before hid half-1: the chunk-0 gate projections are
        # the binding consumer; pass B tolerates hid1 arriving per-tile
        for h in range(1, NHL):
            for d in range(DT):
                ds_ = slice((h * D + d * P), (h * D + (d + 1) * P))
                load(wqgt[h][d][:], wqg_d[ds_, :])
        for d in range(DT):
            ds_ = slice(d * P, (d + 1) * P)
            load(hidh[d][1][:], hid_d[ds_, TH:T])
        load(cost[:, TH:T], cos_d[:, TH:T])
        load(sinpm[:, TH:T], sin_d[:, TH:T])
        for i in range(NHL):
            load(wo[i][:], wo_d[i * P:(i + 1) * P, :])

        ones_col = sbp.tile([P, 1], BF16, tag="ones_col")
        nc.vector.memset(ones_col[:], 1.0)
        ones_row = sbp.tile([1, P], BF16, tag="ones_row")
        nc.vector.memset(ones_row[:], 1.0)
        eps_t = sbp.tile([1, 1], F32, tag="eps_t")
        nc.vector.memset(eps_t[:], EPS)
        ident = sbp.tile([P, P], BF16, tag="ident")
        masks_mod.make_identity(nc, ident[:])
        krot = sbp.tile([P, T], BF16, tag="krot")
        vct = sbp.tile([P, T], BF16, tag="vct")
        vsb = []
        for i in range(KT):
            vsb.append(sbp.tile([P, HD], BF16, tag=f"v{i}", name=f"v{i}"))

        kbfs = []

        # ---- phase 0 pass A: k/v chunks 0-1 + chunk-0 q/gate, d-outer so
        # the PE paces with the hid half-0 DMA stream.
        def proj_pass(half, chunks, qg_psum):
            kps = [pss.tile([P, CT], F32, tag="ss", name=f"kps{c}")
                   for c in chunks]
            vps = [psr.tile([P, CT], F32, tag="row", name=f"vps{c}")
                   for c in chunks]
            for d in range(DT):
                st, sp = (d == 0), (d == DT - 1)
                for i, c in enumerate(chunks):
                    nc.tensor.matmul(kps[i][:], wkT[d][:, :], hidsl(d, c),
                                     start=st, stop=sp)
                    nc.tensor.matmul(vps[i][:], wvT[d][:, :], hidsl(d, c),
                                     start=st, stop=sp)
                if qg_psum is not None:
                    nc.tensor.matmul(qg_psum[0][:], wqqt[0][d][:],
                                     hidsl(d, 0), start=st, stop=sp)
                    nc.tensor.matmul(qg_psum[1][:], wqgt[0][d][:],
                                     hidsl(d, 0), start=st, stop=sp)
            for i, c in enumerate(chunks):
                kbf = sbw.tile([P, CT], BF16, tag="kbf", name="kbf", bufs=4)
                nc.vector.tensor_copy(kbf[:], kps[i][:])
                kbfs.append(kbf)
                nc.vector.tensor_copy(vct[:, c * CT:(c + 1) * CT], vps[i][:])
            for tt in range(chunks[0] * 4, (chunks[-1] + 1) * 4):
                tps = pss.tile([P, P], BF16, tag="ss", name="tps")
                nc.tensor.transpose(tps[:], vct[:, tt * P:(tt + 1) * P],
                                    ident[:])
                nc.vector.tensor_copy(vsb[tt][:], tps[:])

        qp0 = psp.tile([P, CT], F32, tag="pp", name="qp0")
        gp0 = psp.tile([P, CT], F32, tag="pp", name="gp0")
        proj_pass(0, (0, 1), (qp0, gp0))
        q_sb0 = sbq.tile([P, CT], BF16, tag="q_sb", bufs=4, name="q_sb0")
        nc.vector.tensor_copy(q_sb0[:], qp0[:])
        g_sb0 = sbq.tile([P, CT], BF16, tag="g_sb", bufs=5, name="g_sb0")
        nc.vector.tensor_copy(g_sb0[:], gp0[:])
        pre_pairs = {0: (q_sb0, g_sb0)}

        # ---- phase 1: per tq-chunk: q/gate proj, attention ----
        # o_proj for chunk c-1 is emitted after chunk c's norm chains so the
        # PE has dense work while the chains' DVE/ACT latency drains.
        def _o_proj(oc, og, heads=tuple(range(NHL))):
            ocs = slice(oc * CT, (oc + 1) * CT)
            # the final o_proj has the PSUM to itself: rotate all four tag
            # groups so psum-free never gates the accumulation chains
            pools4 = [(psp, "pp"), (pss, "ss"), (psa, "aa"), (psr, "row")]
            for dt in range(DT):
                ds_ = slice(dt * P, (dt + 1) * P)
                pl, tg = pools4[dt % 4]
                pso = pl.tile([P, CT], F32, tag=tg, name="pso")
                for i2, ct4 in enumerate(heads):
                    nc.tensor.matmul(pso[:], wo[ct4][:, ds_], og[ct4][:],
                                     start=(i2 == 0),
                                     stop=(i2 == len(heads) - 1))
                osb = sbw.tile([P, CT], BF16, tag="osb", bufs=4, name="osb")
                if dt % 2 == 0:
                    nc.vector.tensor_copy(osb[:], pso[:])
                else:
                    nc.scalar.copy(osb[:], pso[:])
                rings[dt % 3].dma_start(out_d[ds_, ocs], osb[:])

        prev_gated = None
        for c in range(CH):
            cs = slice(c * CT, (c + 1) * CT)
            q_sbs = []
            g_sbs = []
            sigs = []
            qrots = []

            chain_absr = []

            def _q_chain(h, c=None, cs=None, q_sbs=None, qrots=None):
                qrot = sbw.tile([P, CT], BF16, tag="qrot", bufs=4,
                                name="qrot")
                a = _norm_rope(nc, (sbw, sbr), psr, pss, ones_col[:],
                               ones_row[:], eps_t[:], q_sbs[h][:], qw[:],
                               cost[:, cs], sinpm[:, cs], qrot[:], CT)
                qrots.append(qrot)
                chain_absr.append(a)

            def _proj(w_t, h, dst_tag, bufs_):
                ps = psp.tile([P, CT], F32, tag="pp")
                for d in range(DT):
                    nc.tensor.matmul(ps[:], w_t[h][d][:], hidsl(d, c),
                                     start=(d == 0), stop=(d == DT - 1))
                sb = sbq.tile([P, CT], BF16, tag=dst_tag, bufs=bufs_)
                nc.vector.tensor_copy(sb[:], ps[:])
                return sb

            for h in range(NHL):
                if c == 0:
                    break
                if h == 0 and c in pre_pairs:
                    q_sbs.append(pre_pairs[c][0])
                    g_sbs.append(pre_pairs[c][1])
                    chain_absr.append(_norm_rope(
                        nc, (sbw, sbr), psr, pss, ones_col[:],
                        ones_row[:], eps_t[:], kbfs[c][:], kw[:],
                        cost[:, cs], sinpm[:, cs], krot[:, cs], CT))
                    continue
                ps = psp.tile([P, CT], F32, tag="pp")
                for d in range(DT):
                    nc.tensor.matmul(ps[:], wqqt[h][d][:], hidsl(d, c),
                                     start=(d == 0), stop=(d == DT - 1))
                q_sb = sbq.tile([P, CT], BF16, tag="q_sb", bufs=4)
                nc.vector.tensor_copy(q_sb[:], ps[:])
                q_sbs.append(q_sb)
                ps2 = psp.tile([P, CT], F32, tag="pp")
                for d in range(DT):
                    nc.tensor.matmul(ps2[:], wqgt[h][d][:], hidsl(d, c),
                                     start=(d == 0), stop=(d == DT - 1))
                g_sb = sbq.tile([P, CT], BF16, tag="g_sb", bufs=5)
                nc.vector.tensor_copy(g_sb[:], ps2[:])
                g_sbs.append(g_sb)
                # sandwich a norm/rope chain after each proj pair so the
                # chain's DVE/ACT latency hides behind the next pair's mms
                if h == 0:
                    chain_absr.append(_norm_rope(
                        nc, (sbw, sbr), psr, pss, ones_col[:],
                        ones_row[:], eps_t[:], kbfs[c][:], kw[:],
                        cost[:, cs], sinpm[:, cs], krot[:, cs], CT))
                else:
                    _q_chain(h - 1, c=c, cs=cs, q_sbs=q_sbs, qrots=qrots)
            if c != 0:
                _q_chain(NHL - 1, c=c, cs=cs, q_sbs=q_sbs, qrots=qrots)

            def _emit_sigs():
                for h in range(NHL):
                    # e^-g; the sigmoid is folded into the softmax
                    # denominator: attn*sig(g)/den == attn/((1+e^-g)*den)
                    sig = sbq.tile([P, CT], BF16, tag="sig", bufs=4,
                                   name="sig")
                    si = nc.scalar.activation(sig[:], g_sbs[h][:], AF.Exp,
                                              scale=-1.0)
                    # keep the chunk's absr chain ops contiguous on ACT:
                    # each exp<->absr interleave costs a ~1.3us table load
                    bass._add_dep_helper(si.ins, chain_absr[-1].ins,
                                         sync=False,
                                         reason="group exps after absr")
                    sigs.append(sig)
            gated = []
            nm = 4 * c + 4
            # Filler work drip-fed between attention m-steps keeps the PE
            # dense while ACT runs the exps: o_proj(c-1) tiles, and for
            # chunk 0 (which has no prior o_proj) the next chunk's first
            # projection pair.
            fillers = []
            if prev_gated is not None:
                ocs = slice((c - 1) * CT, c * CT)

                def _mk_oproj(dt, ocs=ocs, og=prev_gated):
                    def run():
                        ds_ = slice(dt * P, (dt + 1) * P)
                        pso = psp.tile([P, CT], F32, tag="pp", name="pso")
                        for ct4 in range(NHL):
                            nc.tensor.matmul(pso[:], wo[ct4][:, ds_],
                                             og[ct4][:], start=(ct4 == 0),
                                             stop=(ct4 == NHL - 1))
                        osb = sbw.tile([P, CT], BF16, tag="osb", bufs=4,
                                       name="osb")
                        nc.vector.tensor_copy(osb[:], pso[:])
                        nc.sync.dma_start(out_d[ds_, ocs], osb[:])
                    return run
                fillers += [_mk_oproj(dt) for dt in range(DT)]
            fill = {"i": 0}
            n_steps = 2 * nm + 6

            def _fill_tick(step):
                due = min(len(fillers),
                          len(fillers) * (step + 1) // n_steps + 1)
                while fill["i"] < due:
                    fillers[fill["i"]]()
                    fill["i"] += 1

            step_no = [0]
            pair_state = {}

            def _attn_mloop(hp):
                pair = (hp, hp + 1)
                attns = {h: psa.tile([P, CT], F32, tag="aa",
                                     name=f"attn{h}") for h in pair}
                esums = {}
                prevE = None
                for m in range(nm):
                    ks = slice(m * P, (m + 1) * P)
                    r = m - 4 * c
                    lo = P * r if r > 0 else 0
                    ns = slice(lo, CT)
                    Es = {}
                    for h in pair:
                        sps = pss.tile([P, CT], F32, tag="ss", name="sps")
                        nc.tensor.matmul(sps[:, ns], krot[:, ks],
                                         qrots[h][:, ns],
                                         start=True, stop=True)
                        E = sbw.tile([P, CT], BF16, tag="E", name="E",
                                     bufs=6)
                        nc.scalar.activation(E[:, ns], sps[:, ns], AF.Exp,
                                             scale=SCALE)
                        if r >= 0:
                            nc.gpsimd.tensor_tensor(
                                E[:, lo:lo + P], E[:, lo:lo + P],
                                masks[:, 0:P], op=ALU.mult)
                        Es[h] = E
                    _fill_tick(step_no[0])
                    step_no[0] += 1
                    for h in pair:
                        # running softmax denominator in SBUF: out-of-place
                        # ping-pong adds keep the DVE in 2x packed mode; the
                        # diagonal (partial-width) steps accumulate in place
                        # to preserve the untouched columns
                        if m == 0 and c == 0:
                            # chunk 0's later steps accumulate in place, so
                            # the running sum must not pin an E-ring slot
                            ne = sbw.tile([P, CT], BF16, tag="esum",
                                          bufs=4, name="esum")
                            nc.vector.tensor_copy(ne[:], Es[h][:])
                            esums[h] = ne
                        elif m == 0:
                            esums[h] = Es[h]
                        elif r > 0:
                            nc.vector.tensor_tensor(
                                esums[h][:, ns], esums[h][:, ns],
                                Es[h][:, ns], op=ALU.add)
                        else:
                            ne = sbw.tile([P, CT], BF16, tag="esum",
                                          bufs=4, name="esum")
                            nc.vector.tensor_tensor(ne[:], esums[h][:],
                                                    Es[h][:], op=ALU.add)
                            esums[h] = ne
                    # PV matmuls run one m-step behind the scores/exp so
                    # each exp has a full step of PE work to hide behind
                    if prevE is not None:
                        pm, pns, pEs = prevE
                        for h in pair:
                            nc.tensor.matmul(attns[h][:, pns], vsb[pm][:],
                                             pEs[h][:, pns],
                                             start=(pm == 0), stop=False)
                    prevE = (m, ns, Es)
                pm, pns, pEs = prevE
                for h in pair:
                    nc.tensor.matmul(attns[h][:, pns], vsb[pm][:],
                                     pEs[h][:, pns], start=(pm == 0),
                                     stop=True)
                _fill_tick(step_no[0])
                step_no[0] += 1
                pair_state[hp] = (attns, esums)

            def _attn_drain(hp):
                pair = (hp, hp + 1)
                attns, esums = pair_state.pop(hp)
                dpss = {}
                for h in pair:
                    dps = psr.tile([1, CT], F32, tag="row", name="dps")
                    nc.tensor.matmul(dps[:], ones_col[:], esums[h][:],
                                     start=True, stop=True)
                    dpss[h] = dps
                _fill_tick(step_no[0])
                step_no[0] += 1
                for h in pair:
                    dn = sbr.tile([1, CT], BF16, tag="rowtmp", name="dn")
                    nc.vector.tensor_copy(dn[:], dpss[h][:])
                    rb = psr.tile([P, CT], F32, tag="row", name="rb")
                    nc.tensor.matmul(rb[:], ones_row[:], dn[:],
                                     start=True, stop=True)
                    gd = sbw.tile([P, CT], F32, tag="rcb", name="gd",
                                  bufs=2)
                    nc.vector.scalar_tensor_tensor(gd[:], sigs[h][:], 1.0,
                                                   rb[:], op0=ALU.add,
                                                   op1=ALU.mult)
                    rcb = sbw.tile([P, CT], F32, tag="rcb", name="rcb",
                                   bufs=2)
                    nc.vector.reciprocal_approx_fast(out=rcb[:], in_=gd[:])
                    g = sbq.tile([P, CT], BF16, tag="gated", bufs=8)
                    nc.vector.tensor_tensor(g[:], attns[h][:], rcb[:],
                                            op=ALU.mult)
                    gated.append(g)
                _fill_tick(step_no[0])
                step_no[0] += 1
                if c == CH - 1 and hp == 0:
                    # heads 0-1's half of this chunk's o_proj can run as
                    # fillers during pair (2,3); host adds out2 to out_t
                    g01 = list(gated)

                    def _mk_ohalf(dt, g01=g01):
                        def run():
                            ds_ = slice(dt * P, (dt + 1) * P)
                            pso = psp.tile([P, CT], F32, tag="pp",
                                           name="psoh")
                            for i2 in range(2):
                                nc.tensor.matmul(pso[:], wo[i2][:, ds_],
                                                 g01[i2][:], start=(i2 == 0),
                                                 stop=(i2 == 1))
                            osb = sbw.tile([P, CT], BF16, tag="osb", bufs=4,
                                           name="osbh")
                            nc.vector.tensor_copy(osb[:], pso[:])
                            nc.sync.dma_start(out2_d[ds_, :], osb[:])
                        return run
                    fillers.extend(_mk_ohalf(dt) for dt in range(DT))

            if c == 0:
                # wqg trails the DMA stripe: all q-projections + chains
                # first (wqq rides the stripe), then pair (0,1) attention
                # runs while the wqg stream lands, then gate projections
                q_sbs.append(pre_pairs[0][0])
                g_sbs.append(pre_pairs[0][1])
                q_sbs.append(_proj(wqqt, 1, "q_sb", 4))
                chain_absr.append(_norm_rope(
                    nc, (sbw, sbr), psr, pss, ones_col[:], ones_row[:],
                    eps_t[:], kbfs[0][:], kw[:], cost[:, cs], sinpm[:, cs],
                    krot[:, cs], CT))
                q_sbs.append(_proj(wqqt, 2, "q_sb", 4))
                _q_chain(0, c=c, cs=cs, q_sbs=q_sbs, qrots=qrots)
                q_sbs.append(_proj(wqqt, 3, "q_sb", 4))
                _q_chain(1, c=c, cs=cs, q_sbs=q_sbs, qrots=qrots)
                _q_chain(2, c=c, cs=cs, q_sbs=q_sbs, qrots=qrots)
                _q_chain(3, c=c, cs=cs, q_sbs=q_sbs, qrots=qrots)
                _attn_mloop(0)
                # phase 0 pass B: k/v for chunks 2-3 from hid half 1 --
                # wqg-independent PE work that covers the gate-weight DMA
                proj_pass(1, (2, 3), None)
                g_sbs.append(_proj(wqgt, 1, "g_sb", 5))
                g_sbs.append(_proj(wqgt, 2, "g_sb", 5))
                g_sbs.append(_proj(wqgt, 3, "g_sb", 5))
                # chunk-1 first-pair prefetch, drip-fed as pair-(2,3) filler
                qp1 = psp.tile([P, CT], F32, tag="pp", name="qp1")
                gp1 = psp.tile([P, CT], F32, tag="pp", name="gp1")

                def _mk_proj(ps_t, w_t, dlist):
                    def run():
                        for d in dlist:
                            nc.tensor.matmul(
                                ps_t[:], w_t[0][d][:], hidsl(d, 1),
                                start=(d == 0), stop=(d == DT - 1))
                    return run
                for d0 in range(0, DT, 4):
                    fillers.append(_mk_proj(qp1, wqqt, range(d0, d0 + 4)))
                for d0 in range(0, DT, 4):
                    fillers.append(_mk_proj(gp1, wqgt, range(d0, d0 + 4)))
                _emit_sigs()
                _attn_drain(0)
                _attn_mloop(2)
                _attn_drain(2)
            else:
                _emit_sigs()
                for hp in (0, 2):
                    _attn_mloop(hp)
                    _attn_drain(hp)

            while fill["i"] < len(fillers):
                fillers[fill["i"]]()
                fill["i"] += 1
            if c == 0:
                q_sb1 = sbq.tile([P, CT], BF16, tag="q_sb", bufs=4,
                                 name="q_sb1")
                nc.vector.tensor_copy(q_sb1[:], qp1[:])
                g_sb1 = sbq.tile([P, CT], BF16, tag="g_sb", bufs=5,
                                 name="g_sb1")
                nc.vector.tensor_copy(g_sb1[:], gp1[:])
                pre_pairs[1] = (q_sb1, g_sb1)
            prev_gated = gated
        _o_proj(CH - 1, prev_gated, heads=(2, 3))
    nc.compile()
    return nc


def make_in_maps(hidden, cos, sin, wq, wk, wv, wo, q_norm_w, k_norm_w):
    """Build the 8 per-core input maps (host-side sharding + layout prep)."""
    i_idx = np.arange(P)[:, None]
    j_idx = np.arange(P)[None, :]
    masks = (j_idx >= i_idx).astype(BF)
    in_maps = []
    for core in range(N_CORES):
        b, g = core // NKV, core % NKV
        heads = range(NHL * g, NHL * g + NHL)
        sin_t = sin[b].T.copy()
        sin_t[:HD // 2] = -sin_t[:HD // 2]
        in_maps.append({
            "hid": np.ascontiguousarray(hidden[b].T).astype(BF),
            "wqq": np.concatenate(
                [wq[:, h * 2 * HD: h * 2 * HD + HD] for h in heads], 0
            ).astype(BF),
            "wqg": np.concatenate(
                [wq[:, h * 2 * HD + HD: (h + 1) * 2 * HD] for h in heads], 0
            ).astype(BF),
            "wk": np.ascontiguousarray(wk[:, g * HD:(g + 1) * HD]).astype(BF),
            "wv": np.ascontiguousarray(wv[:, g * HD:(g + 1) * HD]).astype(BF),
            "wo": np.ascontiguousarray(
                wo[NHL * HD * g: NHL * HD * (g + 1), :]).astype(BF),
            "cost": np.ascontiguousarray(cos[b].T).astype(BF),
            "sinpm": np.ascontiguousarray(sin_t).astype(BF),
            "qw": np.ascontiguousarray(q_norm_w[:, None]).astype(np.float32),
            "kw": np.ascontiguousarray(k_norm_w[:, None]).astype(np.float32),
            "masks": np.ascontiguousarray(masks),
        })
    return in_maps


def _install_ntff_hook():
    """Inject antenv.axon_hooks with a ctypes NTFF profile hook.

    The container's antenv package lacks axon_hooks, so bass_utils'
    trace=True path can't find the hook. Replicates the boot script's
    _ntff_profile_via_ctypes against libaxon_pjrt.so.
    """
    import contextlib
    import ctypes
    import types

    if "antenv.axon_hooks" in sys.modules:
        return
    lib = None
    for so_path in ("/opt/axon/libaxon_pjrt.so",
                    "/root/.axon_site/axon/libaxon_pjrt.so"):
        try:
            lib = ctypes.CDLL(so_path)
            break
        except OSError:
            continue
    if lib is None:
        return
    if not hasattr(lib, "axon_start_nrt_profile"):
        return
    lib.axon_start_nrt_profile.argtypes = [ctypes.POINTER(ctypes.c_int64),
                                           ctypes.c_size_t]
    lib.axon_start_nrt_profile.restype = ctypes.c_int64
    lib.axon_stop_nrt_profile.argtypes = [ctypes.c_char_p]
    lib.axon_stop_nrt_profile.restype = ctypes.c_int64

    @contextlib.contextmanager
    def _hook(output_dir, device_ids):
        import jax

        jax.devices()
        if device_ids:
            ids = (ctypes.c_int64 * len(device_ids))(*device_ids)
            rc = lib.axon_start_nrt_profile(ids, len(device_ids))
        else:
            rc = lib.axon_start_nrt_profile(None, 0)
        if rc != 0:
            raise RuntimeError(f"axon_start_nrt_profile rc={rc}")
        try:
            yield
        finally:
            n = lib.axon_stop_nrt_profile(str(output_dir).encode())
            print(f"profile: {n} file(s) written to {output_dir}",
                  file=sys.stderr)

    m = types.ModuleType("antenv.axon_hooks")
    m.get_axon_ntff_profile_hook = lambda: _hook
    m.set_axon_ntff_profile_hook = lambda h: None
    sys.modules["antenv.axon_hooks"] = m


_NC_CACHE = None


def _get_nc():
    global _NC_CACHE
    if _NC_CACHE is None:
        _NC_CACHE = build_nc()
    return _NC_CACHE


def kernel(hidden_BTD, cos_BTK, sin_BTK, wq, wk, wv, wo, q_norm_w, k_norm_w,
           segment_ids_BT=None, position_ids_BT=None, **_unused):
    from concourse.bass_utils import run_bass_kernel_spmd

    in_maps = make_in_maps(
        np.asarray(hidden_BTD, np.float32), np.asarray(cos_BTK, np.float32),
        np.asarray(sin_BTK, np.float32), np.asarray(wq, np.float32),
        np.asarray(wk, np.float32), np.asarray(wv, np.float32),
        np.asarray(wo, np.float32), np.asarray(q_norm_w, np.float32),
        np.asarray(k_norm_w, np.float32))
    nc = _get_nc()
    trace = bool(int(os.environ.get("BASS_KERNEL_TRACE", "0")))
    if trace:
        _install_ntff_hook()
    res = run_bass_kernel_spmd(nc, in_maps, core_ids=list(range(N_CORES)),
                               trace=trace)
    out = np.zeros((B, T, D), np.float32)
    for core in range(N_CORES):
        out[core // NKV] += res.results[core]["out_t"].astype(np.float32).T
        out[core // NKV, (CH - 1) * CT:] += \
            res.results[core]["out2"].astype(np.float32).T
    kernel.last_exec_time_ns = res.exec_time_ns
    kernel.last_results = res
    return out


kernel.last_exec_time_ns = None
kernel.last_results = None
